# revision 1
# baseline (speedup 1.0000x reference)
"""CARAFE-naive 2x content-aware upsampling on 8 Trainium2 NeuronCores.

Problem: features [2, 256, 100, 100] f32, masks [2, 25, 200, 200] f32
-> out [2, 256, 200, 200] f32, where each output pixel is a 25-tap (5x5)
weighted sum of the source neighborhood, weights shared across channels.

Strategy (per core = one (image n, row-quarter q) pair):
  The 25-tap contraction is cast as TensorE matmuls via a banded-matrix
  trick along the width axis. For one low-res output row h and width
  block of L=50 low-res columns, the contraction over the 5 horizontal
  taps is a matmul with contraction dim K = L+4 = 54 (the padded width
  window): out[c, (a, w2)] = sum_w' F[w', c] * Band[w', (a, w2)], where
  Band packs mask values on 5 diagonals (built host-side in numpy).
  The 5 vertical taps (dy) accumulate in PSUM across 5 matmuls.

  lhsT = transposed feature row slices (stationary), rhs = banded mask
  blocks. Both fp16 (PE runs fp16 at full rate; ~2^-11 rel precision).
  Both width blocks live on SBUF partitions [0, 54) with the block index
  in the free dim -- all matmuls use tile_position (0,0); mixing row
  bases within one PSUM accumulation group crashes the device.

Host-side numpy does layout/packing only (transpose, pad, diagonal
scatter of masks into band matrices); all FLOPs run on the device.
"""

import numpy as np

import concourse.mybir as mybir
import concourse.tile as tile
from concourse import bacc
from concourse.bass_utils import run_bass_kernel_spmd

# problem constants
N, C, H, W = 2, 256, 100, 100
KS = 5        # kernel size
S = 2         # upsample scale
R = (KS - 1) // 2

# sharding / blocking constants
HC = H // 4       # 25 low-res rows per core (8 cores = 2 images x 4 quarters)
NR = HC + 2 * R   # 29 padded feature rows per core
NBLK = 2          # width blocks
L = W // NBLK     # 50 low-res columns per block
KB = L + KS - 1   # 54 = matmul contraction size
PBASE = 64        # SBUF partition base stride between blocks
NCOL = 2 * S * L  # 200 matmul N per block: (a in 2, w2l in 100)
F16 = mybir.dt.float16
F32 = mybir.dt.float32


def build_program(iters: int = 1, dt=F16, blks=(0, 1), copy_eng="both", parts="full",
                  in_chunks: int = 1, in_engines=("sync",)):
    """Build the per-core bass program. `iters`>1 wraps the whole compute in
    a hardware loop (used only for benchmarking slope timing)."""
    nc = bacc.Bacc(None, target_bir_lowering=False, debug=False)
    f_in = nc.dram_tensor("f", [KB, NBLK, NR, C], dt, kind="ExternalInput")
    b_in = nc.dram_tensor("b", [KB, NBLK, HC, KS, NCOL], dt, kind="ExternalInput")
    out = nc.dram_tensor("out", [C, S * HC, S * W], F32, kind="ExternalOutput")

    with tile.TileContext(nc) as tc:
        with (
            tc.tile_pool(name="fsb", bufs=1) as fpool,
            tc.tile_pool(name="bsb", bufs=1) as bpool,
            tc.tile_pool(name="osb", bufs=4) as opool,
            tc.tile_pool(name="ps", bufs=6, space="PSUM") as pspool,
        ):
            def body(_=None):
                F_sb = fpool.tile([KB, NBLK, NR, C], dt)
                B_sb = bpool.tile([KB, NBLK, HC, KS, NCOL], dt)
                if parts == "dmain128":
                    # DMA-bandwidth probe: same bytes, 108-partition layout
                    F2 = fpool.tile([KB * NBLK, NR, C], dt, name="F2")
                    B2 = bpool.tile([KB * NBLK, HC, KS, NCOL], dt, name="B2")
                    f2 = f_in[:].rearrange("k n r c -> (k n) r c")
                    b2 = b_in[:].rearrange("k n h d c -> (k n) h d c")
                    engs = [getattr(nc, e) for e in in_engines]
                    step = (KB * NBLK + in_chunks - 1) // in_chunks
                    for i, p0 in enumerate(range(0, KB * NBLK, step)):
                        p1 = min(p0 + step, KB * NBLK)
                        engs[i % len(engs)].dma_start(F2[p0:p1], f2[p0:p1])
                        engs[i % len(engs)].dma_start(B2[p0:p1], b2[p0:p1])
                    return
                if parts != "nodmain":
                    engs = [getattr(nc, e) for e in in_engines]
                    ei = 0
                    # split each input DMA into in_chunks along a free dim to
                    # engage more DMA queues in parallel
                    fstep = (NR + in_chunks - 1) // in_chunks
                    for r0 in range(0, NR, fstep):
                        r1 = min(r0 + fstep, NR)
                        engs[ei % len(engs)].dma_start(
                            F_sb[:, :, r0:r1], f_in[:, :, r0:r1]
                        )
                        ei += 1
                    bstep = (HC + in_chunks - 1) // in_chunks
                    for h0 in range(0, HC, bstep):
                        h1 = min(h0 + bstep, HC)
                        engs[ei % len(engs)].dma_start(
                            B_sb[:, :, h0:h1], b_in[:, :, h0:h1]
                        )
                        ei += 1
                if parts == "dmain":
                    return
                for ct in range(2):
                    psums = {}
                    for r in range(NR):
                        for blk in blks:
                            lhsT = F_sb[:, blk, r, ct * 128 : (ct + 1) * 128]
                            for dy in range(KS):
                                h = r - dy
                                if not (0 <= h < HC):
                                    continue
                                if dy == 0 and blk == blks[0]:
                                    psums[h] = pspool.tile(
                                        [128, NBLK * NCOL],
                                        F32,
                                        name=f"ps{ct}_{h}",
                                        tag="ps",
                                    )
                                # One accumulation group per PSUM bank: start
                                # zeroes the whole 2KB zero-region, so only
                                # the first matmul of the tile starts and only
                                # the last one stops.
                                nc.tensor.matmul(
                                    psums[h][:, blk * NCOL : (blk + 1) * NCOL],
                                    lhsT,
                                    B_sb[:, blk, h, dy, :],
                                    start=(dy == 0 and blk == blks[0]),
                                    stop=(dy == KS - 1 and blk == blks[-1]),
                                )
                        h_done = r - (KS - 1)
                        if h_done >= 0 and parts in ("full", "nodmain"):
                            ps = psums.pop(h_done)
                            osb = opool.tile([128, 2, NBLK, S * L], F32)
                            # psum free layout (blk, a, w2l) -> (a, blk, w2l)
                            src = ps[:].rearrange(
                                "p (k a w) -> p a k w", k=NBLK, a=2
                            )
                            if copy_eng == "vector" or (copy_eng == "both" and h_done % 2 == 0):
                                nc.vector.tensor_copy(osb[:], src)
                            else:
                                nc.scalar.copy(osb[:], src)
                            nc.sync.dma_start(
                                out[ct * 128 : (ct + 1) * 128,
                                    S * h_done : S * h_done + 2, :],
                                osb[:].rearrange("p a k w -> p a (k w)"),
                            )

            if iters == 1:
                body()
            else:
                with tc.For_i(0, iters, 1) as _i:
                    body(_i)
    nc.finalize()
    return nc


def host_prep(features: np.ndarray, masks: np.ndarray):
    """Pack per-core fp16 inputs: transposed padded feature rows and banded
    mask matrices. Pure layout work (no arithmetic beyond dtype cast)."""
    f_hosts, b_hosts = [], []
    padded = np.pad(features, ((0, 0), (0, 0), (R, R), (R, R)))  # [N,C,H+4,W+4]
    wl_idx = np.arange(L)
    for core in range(8):
        n, q = divmod(core, 4)
        h0 = HC * q
        F_core = padded[n, :, h0 : h0 + NR, :]  # [C, 29, 104]
        F_host = np.zeros((KB, NBLK, NR, C), np.float16)
        for blk in range(NBLK):
            F_host[:, blk] = F_core[:, :, L * blk : L * blk + KB].transpose(2, 1, 0)
        # masks[n]: [25, 200, 200] -> [dy, dx, h, a, w, b]
        m7 = masks[n].reshape(KS, KS, H, S, W, S)[:, :, h0 : h0 + HC]
        B_host = np.zeros((KB, NBLK, HC, KS, 2, L, 2), np.float16)
        for blk in range(NBLK):
            for dx in range(KS):
                src = m7[:, dx, :, :, L * blk : L * blk + L, :]  # [dy,h,a,wl,b]
                B_host[dx + wl_idx, blk, :, :, :, wl_idx, :] = (
                    src.transpose(3, 1, 0, 2, 4)
                )
        f_hosts.append(F_host)
        b_hosts.append(B_host.reshape(KB, NBLK, HC, KS, NCOL))
    return f_hosts, b_hosts


# ---------------- v2: 128-partition layout, per-block PSUM banks ----------------
KB2 = 64  # padded contraction size (54 useful + 10 zero rows) -> blocks at 0/64


def build_program_v2(iters: int = 1, dt=F16, copy_eng="both", parts="full",
                     psbufs: int = 3, obufs: int = 2, out_group: int = 5,
                     bchunks: int = 5):
    """v2: both width blocks packed on 128 partitions (bases 0/64), each block
    accumulating into its own PSUM bank (documented-safe row-tiling pattern).
    dy-inner loop: weights reload per matmul but the two block chains run
    concurrently on different PE row groups."""
    nc = bacc.Bacc(None, target_bir_lowering=False, debug=False)
    f_in = nc.dram_tensor("f", [128, NR, C], dt, kind="ExternalInput")
    b_in = nc.dram_tensor("b", [128, HC, KS, NCOL], dt, kind="ExternalInput")
    out = nc.dram_tensor("out", [C, S * HC, S * W], F32, kind="ExternalOutput")

    with tile.TileContext(nc) as tc:
        with (
            tc.tile_pool(name="fsb", bufs=1) as fpool,
            tc.tile_pool(name="bsb", bufs=1) as bpool,
            tc.tile_pool(name="osb", bufs=obufs) as opool,
            tc.tile_pool(name="ps0", bufs=psbufs, space="PSUM") as pspool0,
            tc.tile_pool(name="ps1", bufs=psbufs, space="PSUM") as pspool1,
        ):
            pspools = [pspool0, pspool1]

            def body(_=None):
                F_sb = fpool.tile([128, NR, C], dt)
                B_sb = bpool.tile([128, HC, KS, NCOL], dt)
                if parts != "nodmain":
                    # chunked input DMAs: lets matmuls start after chunk 0
                    nc.sync.dma_start(F_sb[:, : NR // 2], f_in[:, : NR // 2])
                    nc.sync.dma_start(F_sb[:, NR // 2 :], f_in[:, NR // 2 :])
                    bstep = (HC + bchunks - 1) // bchunks
                    for h0 in range(0, HC, bstep):
                        h1 = min(h0 + bstep, HC)
                        nc.sync.dma_start(B_sb[:, h0:h1], b_in[:, h0:h1])
                if parts == "dmain":
                    return
                G = out_group
                for ct in range(2):
                    for g0 in range(0, HC, G):
                        g1 = min(g0 + G, HC)
                        osb = opool.tile([128, G, 2, NBLK * S * L], F32)
                        for h in range(g0, g1):
                            ps = [
                                pspools[blk].tile(
                                    [128, NCOL], F32, name=f"ps{blk}_{ct}_{h}",
                                    tag=f"psb{blk}",
                                )
                                for blk in range(NBLK)
                            ]
                            for dy in range(KS):
                                for blk in range(NBLK):
                                    lo = KB2 * blk
                                    nc.tensor.matmul(
                                        ps[blk][:, :],
                                        F_sb[lo : lo + KB2, h + dy,
                                             ct * 128 : (ct + 1) * 128],
                                        B_sb[lo : lo + KB2, h, dy, :],
                                        start=(dy == 0),
                                        stop=(dy == KS - 1),
                                    )
                            if parts == "nocopy":
                                continue
                            # osb free layout per h: (a, blk, w2l) built from the
                            # two psum tiles; dest dims [2, (blk, 100)]
                            dstv = osb[:, h - g0].rearrange(
                                "p a (k w) -> p a k w", k=NBLK
                            )
                            for blk in range(NBLK):
                                src = ps[blk][:].rearrange("p (a w) -> p a w", a=2)
                                dst = dstv[:, :, blk, :]
                                if copy_eng == "vector" or (
                                    copy_eng == "both" and blk == 0
                                ):
                                    nc.vector.tensor_copy(dst, src)
                                else:
                                    nc.scalar.copy(dst, src)
                        if parts == "nocopy":
                            continue
                        nc.sync.dma_start(
                            out[ct * 128 : (ct + 1) * 128,
                                S * g0 : S * g1, :],
                            osb[:, : g1 - g0].rearrange("p g a c -> p (g a c)"),
                        )

            if iters == 1:
                body()
            else:
                with tc.For_i(0, iters, 1) as _i:
                    body(_i)
    nc.finalize()
    return nc


def host_prep_v2(features: np.ndarray, masks: np.ndarray):
    """v2 layouts: [128, ...] with partition = 64*blk + w'' (w'' in [0,54))."""
    f_hosts, b_hosts = [], []
    padded = np.pad(features, ((0, 0), (0, 0), (R, R), (R, R)))
    wl_idx = np.arange(L)
    for core in range(8):
        n, q = divmod(core, 4)
        h0 = HC * q
        F_core = padded[n, :, h0 : h0 + NR, :]  # [C, 29, 104]
        F_host = np.zeros((128, NR, C), np.float16)
        for blk in range(NBLK):
            F_host[KB2 * blk : KB2 * blk + KB] = (
                F_core[:, :, L * blk : L * blk + KB].transpose(2, 1, 0)
            )
        m7 = masks[n].reshape(KS, KS, H, S, W, S)[:, :, h0 : h0 + HC]
        B_host = np.zeros((128, HC, KS, 2, L, 2), np.float16)
        for blk in range(NBLK):
            for dx in range(KS):
                src = m7[:, dx, :, :, L * blk : L * blk + L, :]  # [dy,h,a,wl,b]
                B_host[KB2 * blk + dx + wl_idx, :, :, :, wl_idx, :] = (
                    src.transpose(3, 1, 0, 2, 4)
                )
        f_hosts.append(F_host)
        b_hosts.append(B_host.reshape(128, HC, KS, NCOL))
    return f_hosts, b_hosts


# ---------------- v3: dy-pairs stacked in K (two taps per matmul) ----------------
NP3 = (KS + 1) // 2  # 3 matmuls per (h, blk): dy pairs (0,1), (2,3), (4,-)


def build_program_v3(iters: int = 1, dt=F16, copy_eng="both", parts="full",
                     psbufs: int = 3, obufs: int = 2, out_group: int = 5,
                     bchunks: int = 5, unroll: bool = False):
    """v3: K=128 = (dy-pair half j in {0,1}) x (w'' in [0,64)). The upper 64
    partitions hold a one-row-shifted copy of the features, so one matmul
    contracts two vertical taps. 300 matmuls of N=200, all tile_position
    (0,0), one PSUM bank per output row."""
    nc = bacc.Bacc(None, target_bir_lowering=False, debug=False)
    f_in = nc.dram_tensor("f", [128, NBLK, NR, C], dt, kind="ExternalInput")
    b_in = nc.dram_tensor("b", [128, NBLK, HC, NP3, NCOL], dt, kind="ExternalInput")
    out = nc.dram_tensor("out", [C, S * HC, S * W], F32, kind="ExternalOutput")

    with tile.TileContext(nc) as tc:
        with (
            tc.tile_pool(name="fsb", bufs=1) as fpool,
            tc.tile_pool(name="bsb", bufs=1) as bpool,
            tc.tile_pool(name="osb", bufs=obufs) as opool,
            tc.tile_pool(name="ps", bufs=psbufs, space="PSUM") as pspool,
        ):
            def body(_=None):
                F_sb = fpool.tile([128, NBLK, NR, C], dt)
                B_sb = bpool.tile([128, NBLK, HC, NP3, NCOL], dt)
                if parts != "nodmain":
                    nc.sync.dma_start(F_sb[:, :, : NR // 2], f_in[:, :, : NR // 2])
                    nc.sync.dma_start(F_sb[:, :, NR // 2 :], f_in[:, :, NR // 2 :])
                    bstep = (HC + bchunks - 1) // bchunks
                    for h0 in range(0, HC, bstep):
                        h1 = min(h0 + bstep, HC)
                        nc.sync.dma_start(B_sb[:, :, h0:h1], b_in[:, :, h0:h1])
                if parts == "dmain":
                    return
                G = out_group
                for ct in range(2):
                    for g0 in range(0, HC, G):
                        g1 = min(g0 + G, HC)
                        osb = opool.tile([128, G, 2, NBLK * S * L], F32)
                        for h in range(g0, g1):
                            ps = pspool.tile(
                                [128, NBLK * NCOL], F32, name=f"ps_{ct}_{h}",
                                tag="ps",
                            )
                            for blk in range(NBLK):
                                for p in range(NP3):
                                    nc.tensor.matmul(
                                        ps[:, blk * NCOL : (blk + 1) * NCOL],
                                        F_sb[:, blk, h + 2 * p,
                                             ct * 128 : (ct + 1) * 128],
                                        B_sb[:, blk, h, p, :],
                                        start=(blk == 0 and p == 0),
                                        stop=(blk == NBLK - 1 and p == NP3 - 1),
                                    )
                            if parts == "nocopy":
                                continue
                            # psum free layout (blk, a, w2l) -> dest (a, blk, w2l)
                            src = ps[:].rearrange("p (k a w) -> p a k w", k=NBLK, a=2)
                            dst = osb[:, h - g0].rearrange(
                                "p a (k w) -> p a k w", k=NBLK
                            )
                            if copy_eng == "vector" or (
                                copy_eng == "both" and h % 2 == 0
                            ):
                                nc.vector.tensor_copy(dst, src)
                            else:
                                nc.scalar.copy(dst, src)
                        if parts == "nocopy":
                            continue
                        nc.sync.dma_start(
                            out[ct * 128 : (ct + 1) * 128, S * g0 : S * g1, :],
                            osb[:, : g1 - g0].rearrange("p g a c -> p (g a c)"),
                        )

            if iters == 1:
                body()
            elif unroll:
                for _k in range(iters):
                    body(_k)
            else:
                with tc.For_i(0, iters, 1) as _i:
                    body(_i)
    nc.finalize()
    return nc


def host_prep_v3(features: np.ndarray, masks: np.ndarray):
    """v3 layouts: partition = 64*j + w''; j=1 half holds features shifted one
    row down (dy-pair trick). Separate windows per width block."""
    f_hosts, b_hosts = [], []
    padded = np.pad(features, ((0, 0), (0, 0), (R, R), (R, R)))
    wl_idx = np.arange(L)
    for core in range(8):
        n, q = divmod(core, 4)
        h0 = HC * q
        F_core = padded[n, :, h0 : h0 + NR, :]  # [C, 29, 104]
        F_host = np.zeros((128, NBLK, NR, C), np.float16)
        for blk in range(NBLK):
            win = F_core[:, :, L * blk : L * blk + KB].transpose(2, 1, 0)  # [54,29,C]
            F_host[:KB, blk] = win                      # j=0: rows r
            F_host[64 : 64 + KB, blk, : NR - 1] = win[:, 1:]  # j=1: rows r+1
        m7 = masks[n].reshape(KS, KS, H, S, W, S)[:, :, h0 : h0 + HC]
        B_host = np.zeros((128, NBLK, HC, NP3, 2, L, 2), np.float16)
        for blk in range(NBLK):
            for dx in range(KS):
                for dy in range(KS):
                    p, j = divmod(dy, 2)
                    src = m7[dy, dx, :, :, L * blk : L * blk + L, :]  # [h,a,wl,b]
                    B_host[64 * j + dx + wl_idx, blk, :, p, :, wl_idx, :] = (
                        src.transpose(2, 0, 1, 3)
                    )
        f_hosts.append(F_host)
        b_hosts.append(B_host.reshape(128, NBLK, HC, NP3, NCOL))
    return f_hosts, b_hosts


_NC_CACHE = {}


def _get_program(iters: int = 1):
    # v2 is the best HW-verified configuration: 128-partition input layouts
    # (~270 GB/s vs ~60 GB/s for partial-partition DMAs), per-block PSUM
    # banks, batched 5-row output DMAs. Measured 348 us/core steady-state,
    # relative error 1.9e-3 vs the fp32 reference.
    if iters not in _NC_CACHE:
        _NC_CACHE[iters] = build_program_v2(iters)
    return _NC_CACHE[iters]


def kernel(features: np.ndarray, masks: np.ndarray) -> np.ndarray:
    features = np.ascontiguousarray(features, dtype=np.float32)
    masks = np.ascontiguousarray(masks, dtype=np.float32)
    f_hosts, b_hosts = host_prep_v2(features, masks)
    in_maps = [{"f": f_hosts[c], "b": b_hosts[c]} for c in range(8)]
    nc = _get_program(1)
    res = run_bass_kernel_spmd(nc, in_maps, list(range(8)))
    out = np.empty((N, C, S * H, S * W), np.float32)
    for core in range(8):
        n, q = divmod(core, 4)
        out[n, :, S * HC * q : S * HC * (q + 1), :] = res.results[core]["out"]
    return out



# revision 17
# speedup vs baseline: 1.2912x; 1.2912x over previous
"""CARAFE-naive 2x content-aware upsampling on 8 Trainium2 NeuronCores.

Problem: features [2, 256, 100, 100] f32, masks [2, 25, 200, 200] f32
-> out [2, 256, 200, 200] f32, where each output pixel is a 25-tap (5x5)
weighted sum of the source neighborhood, weights shared across channels.

Strategy (per core = one (image n, row-quarter q) pair):
  The 25-tap contraction is cast as TensorE matmuls via a banded-matrix
  trick along the width axis. For one low-res output row h and width
  block of L=50 low-res columns, the contraction over the 5 horizontal
  taps is a matmul with contraction dim K = L+4 = 54 (the padded width
  window): out[c, (a, w2)] = sum_w' F[w', c] * Band[w', (a, w2)], where
  Band packs mask values on 5 diagonals (built host-side in numpy).
  The 5 vertical taps (dy) accumulate in PSUM across 5 matmuls.

  lhsT = transposed feature row slices (stationary), rhs = banded mask
  blocks. Both fp16 (PE runs fp16 at full rate; ~2^-11 rel precision).
  Both width blocks live on SBUF partitions [0, 54) with the block index
  in the free dim -- all matmuls use tile_position (0,0); mixing row
  bases within one PSUM accumulation group crashes the device.

Host-side numpy does layout/packing only (transpose, pad, diagonal
scatter of masks into band matrices); all FLOPs run on the device.
"""

import numpy as np

import concourse.mybir as mybir
import concourse.tile as tile
from concourse import bacc
from concourse.bass_utils import run_bass_kernel_spmd

# problem constants
N, C, H, W = 2, 256, 100, 100
KS = 5        # kernel size
S = 2         # upsample scale
R = (KS - 1) // 2

# sharding / blocking constants
HC = H // 4       # 25 low-res rows per core (8 cores = 2 images x 4 quarters)
NR = HC + 2 * R   # 29 padded feature rows per core
NBLK = 2          # width blocks
L = W // NBLK     # 50 low-res columns per block
KB = L + KS - 1   # 54 = matmul contraction size
PBASE = 64        # SBUF partition base stride between blocks
NCOL = 2 * S * L  # 200 matmul N per block: (a in 2, w2l in 100)
F16 = mybir.dt.float16
F32 = mybir.dt.float32


def build_program(iters: int = 1, dt=F16, blks=(0, 1), copy_eng="both", parts="full",
                  in_chunks: int = 1, in_engines=("sync",)):
    """Build the per-core bass program. `iters`>1 wraps the whole compute in
    a hardware loop (used only for benchmarking slope timing)."""
    nc = bacc.Bacc(None, target_bir_lowering=False, debug=False)
    f_in = nc.dram_tensor("f", [KB, NBLK, NR, C], dt, kind="ExternalInput")
    b_in = nc.dram_tensor("b", [KB, NBLK, HC, KS, NCOL], dt, kind="ExternalInput")
    out = nc.dram_tensor("out", [C, S * HC, S * W], F32, kind="ExternalOutput")

    with tile.TileContext(nc) as tc:
        with (
            tc.tile_pool(name="fsb", bufs=1) as fpool,
            tc.tile_pool(name="bsb", bufs=1) as bpool,
            tc.tile_pool(name="osb", bufs=4) as opool,
            tc.tile_pool(name="ps", bufs=6, space="PSUM") as pspool,
        ):
            def body(_=None):
                F_sb = fpool.tile([KB, NBLK, NR, C], dt)
                B_sb = bpool.tile([KB, NBLK, HC, KS, NCOL], dt)
                if parts == "dmain128":
                    # DMA-bandwidth probe: same bytes, 108-partition layout
                    F2 = fpool.tile([KB * NBLK, NR, C], dt, name="F2")
                    B2 = bpool.tile([KB * NBLK, HC, KS, NCOL], dt, name="B2")
                    f2 = f_in[:].rearrange("k n r c -> (k n) r c")
                    b2 = b_in[:].rearrange("k n h d c -> (k n) h d c")
                    engs = [getattr(nc, e) for e in in_engines]
                    step = (KB * NBLK + in_chunks - 1) // in_chunks
                    for i, p0 in enumerate(range(0, KB * NBLK, step)):
                        p1 = min(p0 + step, KB * NBLK)
                        engs[i % len(engs)].dma_start(F2[p0:p1], f2[p0:p1])
                        engs[i % len(engs)].dma_start(B2[p0:p1], b2[p0:p1])
                    return
                if parts != "nodmain":
                    engs = [getattr(nc, e) for e in in_engines]
                    ei = 0
                    # split each input DMA into in_chunks along a free dim to
                    # engage more DMA queues in parallel
                    fstep = (NR + in_chunks - 1) // in_chunks
                    for r0 in range(0, NR, fstep):
                        r1 = min(r0 + fstep, NR)
                        engs[ei % len(engs)].dma_start(
                            F_sb[:, :, r0:r1], f_in[:, :, r0:r1]
                        )
                        ei += 1
                    bstep = (HC + in_chunks - 1) // in_chunks
                    for h0 in range(0, HC, bstep):
                        h1 = min(h0 + bstep, HC)
                        engs[ei % len(engs)].dma_start(
                            B_sb[:, :, h0:h1], b_in[:, :, h0:h1]
                        )
                        ei += 1
                if parts == "dmain":
                    return
                for ct in range(2):
                    psums = {}
                    for r in range(NR):
                        for blk in blks:
                            lhsT = F_sb[:, blk, r, ct * 128 : (ct + 1) * 128]
                            for dy in range(KS):
                                h = r - dy
                                if not (0 <= h < HC):
                                    continue
                                if dy == 0 and blk == blks[0]:
                                    psums[h] = pspool.tile(
                                        [128, NBLK * NCOL],
                                        F32,
                                        name=f"ps{ct}_{h}",
                                        tag="ps",
                                    )
                                # One accumulation group per PSUM bank: start
                                # zeroes the whole 2KB zero-region, so only
                                # the first matmul of the tile starts and only
                                # the last one stops.
                                nc.tensor.matmul(
                                    psums[h][:, blk * NCOL : (blk + 1) * NCOL],
                                    lhsT,
                                    B_sb[:, blk, h, dy, :],
                                    start=(dy == 0 and blk == blks[0]),
                                    stop=(dy == KS - 1 and blk == blks[-1]),
                                )
                        h_done = r - (KS - 1)
                        if h_done >= 0 and parts in ("full", "nodmain"):
                            ps = psums.pop(h_done)
                            osb = opool.tile([128, 2, NBLK, S * L], F32)
                            # psum free layout (blk, a, w2l) -> (a, blk, w2l)
                            src = ps[:].rearrange(
                                "p (k a w) -> p a k w", k=NBLK, a=2
                            )
                            if copy_eng == "vector" or (copy_eng == "both" and h_done % 2 == 0):
                                nc.vector.tensor_copy(osb[:], src)
                            else:
                                nc.scalar.copy(osb[:], src)
                            nc.sync.dma_start(
                                out[ct * 128 : (ct + 1) * 128,
                                    S * h_done : S * h_done + 2, :],
                                osb[:].rearrange("p a k w -> p a (k w)"),
                            )

            if iters == 1:
                body()
            else:
                with tc.For_i(0, iters, 1) as _i:
                    body(_i)
    nc.finalize()
    return nc


def host_prep(features: np.ndarray, masks: np.ndarray):
    """Pack per-core fp16 inputs: transposed padded feature rows and banded
    mask matrices. Pure layout work (no arithmetic beyond dtype cast)."""
    f_hosts, b_hosts = [], []
    padded = np.pad(features, ((0, 0), (0, 0), (R, R), (R, R)))  # [N,C,H+4,W+4]
    wl_idx = np.arange(L)
    for core in range(8):
        n, q = divmod(core, 4)
        h0 = HC * q
        F_core = padded[n, :, h0 : h0 + NR, :]  # [C, 29, 104]
        F_host = np.zeros((KB, NBLK, NR, C), np.float16)
        for blk in range(NBLK):
            F_host[:, blk] = F_core[:, :, L * blk : L * blk + KB].transpose(2, 1, 0)
        # masks[n]: [25, 200, 200] -> [dy, dx, h, a, w, b]
        m7 = masks[n].reshape(KS, KS, H, S, W, S)[:, :, h0 : h0 + HC]
        B_host = np.zeros((KB, NBLK, HC, KS, 2, L, 2), np.float16)
        for blk in range(NBLK):
            for dx in range(KS):
                src = m7[:, dx, :, :, L * blk : L * blk + L, :]  # [dy,h,a,wl,b]
                B_host[dx + wl_idx, blk, :, :, :, wl_idx, :] = (
                    src.transpose(3, 1, 0, 2, 4)
                )
        f_hosts.append(F_host)
        b_hosts.append(B_host.reshape(KB, NBLK, HC, KS, NCOL))
    return f_hosts, b_hosts


# ---------------- v2: 128-partition layout, per-block PSUM banks ----------------
KB2 = 64  # padded contraction size (54 useful + 10 zero rows) -> blocks at 0/64


def build_program_v2(iters: int = 1, dt=F16, copy_eng="both", parts="full",
                     psbufs: int = 3, obufs: int = 2, out_group: int = 5,
                     bchunks: int = 5):
    """v2: both width blocks packed on 128 partitions (bases 0/64), each block
    accumulating into its own PSUM bank (documented-safe row-tiling pattern).
    dy-inner loop: weights reload per matmul but the two block chains run
    concurrently on different PE row groups."""
    nc = bacc.Bacc(None, target_bir_lowering=False, debug=False)
    f_in = nc.dram_tensor("f", [128, NR, C], dt, kind="ExternalInput")
    b_in = nc.dram_tensor("b", [128, HC, KS, NCOL], dt, kind="ExternalInput")
    out = nc.dram_tensor("out", [C, S * HC, S * W], F32, kind="ExternalOutput")

    with tile.TileContext(nc) as tc:
        with (
            tc.tile_pool(name="fsb", bufs=1) as fpool,
            tc.tile_pool(name="bsb", bufs=1) as bpool,
            tc.tile_pool(name="osb", bufs=obufs) as opool,
            tc.tile_pool(name="ps0", bufs=psbufs, space="PSUM") as pspool0,
            tc.tile_pool(name="ps1", bufs=psbufs, space="PSUM") as pspool1,
        ):
            pspools = [pspool0, pspool1]

            def body(_=None):
                F_sb = fpool.tile([128, NR, C], dt)
                B_sb = bpool.tile([128, HC, KS, NCOL], dt)
                if parts != "nodmain":
                    # chunked input DMAs: lets matmuls start after chunk 0
                    nc.sync.dma_start(F_sb[:, : NR // 2], f_in[:, : NR // 2])
                    nc.sync.dma_start(F_sb[:, NR // 2 :], f_in[:, NR // 2 :])
                    bstep = (HC + bchunks - 1) // bchunks
                    for h0 in range(0, HC, bstep):
                        h1 = min(h0 + bstep, HC)
                        nc.sync.dma_start(B_sb[:, h0:h1], b_in[:, h0:h1])
                if parts == "dmain":
                    return
                G = out_group
                for ct in range(2):
                    for g0 in range(0, HC, G):
                        g1 = min(g0 + G, HC)
                        osb = opool.tile([128, G, 2, NBLK * S * L], F32)
                        for h in range(g0, g1):
                            ps = [
                                pspools[blk].tile(
                                    [128, NCOL], F32, name=f"ps{blk}_{ct}_{h}",
                                    tag=f"psb{blk}",
                                )
                                for blk in range(NBLK)
                            ]
                            for dy in range(KS):
                                for blk in range(NBLK):
                                    lo = KB2 * blk
                                    nc.tensor.matmul(
                                        ps[blk][:, :],
                                        F_sb[lo : lo + KB2, h + dy,
                                             ct * 128 : (ct + 1) * 128],
                                        B_sb[lo : lo + KB2, h, dy, :],
                                        start=(dy == 0),
                                        stop=(dy == KS - 1),
                                    )
                            if parts == "nocopy":
                                continue
                            # osb free layout per h: (a, blk, w2l) built from the
                            # two psum tiles; dest dims [2, (blk, 100)]
                            dstv = osb[:, h - g0].rearrange(
                                "p a (k w) -> p a k w", k=NBLK
                            )
                            for blk in range(NBLK):
                                src = ps[blk][:].rearrange("p (a w) -> p a w", a=2)
                                dst = dstv[:, :, blk, :]
                                if copy_eng == "vector" or (
                                    copy_eng == "both" and blk == 0
                                ):
                                    nc.vector.tensor_copy(dst, src)
                                else:
                                    nc.scalar.copy(dst, src)
                        if parts == "nocopy":
                            continue
                        nc.sync.dma_start(
                            out[ct * 128 : (ct + 1) * 128,
                                S * g0 : S * g1, :],
                            osb[:, : g1 - g0].rearrange("p g a c -> p (g a c)"),
                        )

            if iters == 1:
                body()
            else:
                with tc.For_i(0, iters, 1) as _i:
                    body(_i)
    nc.finalize()
    return nc


def host_prep_v2(features: np.ndarray, masks: np.ndarray):
    """v2 layouts: [128, ...] with partition = 64*blk + w'' (w'' in [0,54))."""
    f_hosts, b_hosts = [], []
    padded = np.pad(features, ((0, 0), (0, 0), (R, R), (R, R)))
    wl_idx = np.arange(L)
    for core in range(8):
        n, q = divmod(core, 4)
        h0 = HC * q
        F_core = padded[n, :, h0 : h0 + NR, :]  # [C, 29, 104]
        F_host = np.zeros((128, NR, C), np.float16)
        for blk in range(NBLK):
            F_host[KB2 * blk : KB2 * blk + KB] = (
                F_core[:, :, L * blk : L * blk + KB].transpose(2, 1, 0)
            )
        m7 = masks[n].reshape(KS, KS, H, S, W, S)[:, :, h0 : h0 + HC]
        B_host = np.zeros((128, HC, KS, 2, L, 2), np.float16)
        for blk in range(NBLK):
            for dx in range(KS):
                src = m7[:, dx, :, :, L * blk : L * blk + L, :]  # [dy,h,a,wl,b]
                B_host[KB2 * blk + dx + wl_idx, :, :, :, wl_idx, :] = (
                    src.transpose(3, 1, 0, 2, 4)
                )
        f_hosts.append(F_host)
        b_hosts.append(B_host.reshape(128, HC, KS, NCOL))
    return f_hosts, b_hosts


# ---------------- v4: v2 + fp16 output (halves the out-DMA bytes) ----------------


def build_program_v4(iters: int = 1, dt=F16, parts="full",
                     psbufs: int = 3, obufs: int = 2, out_group: int = 5,
                     bchunks: int = 5):
    """v2 compute structure, but the PSUM->SBUF copy downcasts to fp16 and the
    output DRAM tensor is fp16 (host upcasts). Out-DMA drops 10.24 -> 5.12 MB."""
    nc = bacc.Bacc(None, target_bir_lowering=False, debug=False)
    f_in = nc.dram_tensor("f", [128, NR, C], dt, kind="ExternalInput")
    b_in = nc.dram_tensor("b", [128, HC, KS, NCOL], dt, kind="ExternalInput")
    out = nc.dram_tensor("out", [C, S * HC, S * W], F16, kind="ExternalOutput")

    with tile.TileContext(nc) as tc:
        with (
            tc.tile_pool(name="fsb", bufs=1) as fpool,
            tc.tile_pool(name="bsb", bufs=1) as bpool,
            tc.tile_pool(name="osb", bufs=obufs) as opool,
            tc.tile_pool(name="ps0", bufs=psbufs, space="PSUM") as pspool0,
            tc.tile_pool(name="ps1", bufs=psbufs, space="PSUM") as pspool1,
        ):
            pspools = [pspool0, pspool1]

            def body(_=None):
                F_sb = fpool.tile([128, NR, C], dt)
                B_sb = bpool.tile([128, HC, KS, NCOL], dt)
                if parts != "nodmain":
                    nc.sync.dma_start(F_sb[:, : NR // 2], f_in[:, : NR // 2])
                    nc.sync.dma_start(F_sb[:, NR // 2 :], f_in[:, NR // 2 :])
                    bstep = (HC + bchunks - 1) // bchunks
                    for h0 in range(0, HC, bstep):
                        h1 = min(h0 + bstep, HC)
                        nc.sync.dma_start(B_sb[:, h0:h1], b_in[:, h0:h1])
                if parts == "dmain":
                    return
                G = out_group
                for ct in range(2):
                    for g0 in range(0, HC, G):
                        g1 = min(g0 + G, HC)
                        osb = opool.tile([128, G, 2, NBLK * S * L], F16)
                        for h in range(g0, g1):
                            ps = [
                                pspools[blk].tile(
                                    [128, NCOL], F32, name=f"ps{blk}_{ct}_{h}",
                                    tag=f"psb{blk}",
                                )
                                for blk in range(NBLK)
                            ]
                            for dy in range(KS):
                                for blk in range(NBLK):
                                    lo = KB2 * blk
                                    nc.tensor.matmul(
                                        ps[blk][:, :],
                                        F_sb[lo : lo + KB2, h + dy,
                                             ct * 128 : (ct + 1) * 128],
                                        B_sb[lo : lo + KB2, h, dy, :],
                                        start=(dy == 0),
                                        stop=(dy == KS - 1),
                                    )
                            dstv = osb[:, h - g0].rearrange(
                                "p a (k w) -> p a k w", k=NBLK
                            )
                            for blk in range(NBLK):
                                src = ps[blk][:].rearrange("p (a w) -> p a w", a=2)
                                dst = dstv[:, :, blk, :]
                                if blk == 0:
                                    nc.vector.tensor_copy(dst, src)
                                else:
                                    nc.scalar.copy(dst, src)
                        nc.sync.dma_start(
                            out[ct * 128 : (ct + 1) * 128,
                                S * g0 : S * g1, :],
                            osb[:, : g1 - g0].rearrange("p g a c -> p (g a c)"),
                        )

            if iters == 1:
                body()
            else:
                with tc.For_i(0, iters, 1) as _i:
                    body(_i)
    nc.finalize()
    return nc


host_prep_v4 = host_prep_v2


# ---------------- v5: v4 + DMA orchestration for single-shot latency ----------------


def build_program_v5(iters: int = 1, dt=F16, psbufs: int = 4, obufs: int = 3,
                     out_group: int = 5,
                     bsplit=(3, 3, 4, 4, 5, 6), fsplit=(9, 10, 10)):
    """v2 compute + fp16 out + latency-ordered DMAs.

    - h-outer / ct-inner loop: each mask chunk feeds 2x the matmuls, so the
      mask stream only needs half the bandwidth to stay ahead of the PE.
    - b chunks land on the sync HWDGE queue in consumption order (small
      first chunk -> first matmul starts early); f chunks and all output
      DMAs ride the scalar HWDGE queue so outputs never queue behind masks.
    """
    nc = bacc.Bacc(None, target_bir_lowering=False, debug=False)
    f_in = nc.dram_tensor("f", [128, NR, C], dt, kind="ExternalInput")
    b_in = nc.dram_tensor("b", [128, HC, KS, NCOL], dt, kind="ExternalInput")
    out = nc.dram_tensor("out", [C, S * HC, S * W], F16, kind="ExternalOutput")

    with tile.TileContext(nc) as tc:
        with (
            tc.tile_pool(name="fsb", bufs=1) as fpool,
            tc.tile_pool(name="bsb", bufs=1) as bpool,
            tc.tile_pool(name="osb", bufs=obufs) as opool,
            tc.tile_pool(name="ps0", bufs=psbufs, space="PSUM") as pspool0,
            tc.tile_pool(name="ps1", bufs=psbufs, space="PSUM") as pspool1,
        ):
            pspools = [pspool0, pspool1]

            def body(_=None):
                F_sb = fpool.tile([128, NR, C], dt)
                B_sb = bpool.tile([128, HC, KS, NCOL], dt)
                # interleave issue order: B0, F0 first (gates the first MM)
                h0 = 0
                nc.sync.dma_start(B_sb[:, : bsplit[0]], b_in[:, : bsplit[0]])
                r0 = 0
                nc.scalar.dma_start(
                    F_sb[:, : fsplit[0]], f_in[:, : fsplit[0]]
                )
                h0 += bsplit[0]
                for bs in bsplit[1:]:
                    nc.sync.dma_start(B_sb[:, h0 : h0 + bs], b_in[:, h0 : h0 + bs])
                    h0 += bs
                r0 += fsplit[0]
                for fs in fsplit[1:]:
                    nc.scalar.dma_start(
                        F_sb[:, r0 : r0 + fs], f_in[:, r0 : r0 + fs]
                    )
                    r0 += fs
                G = out_group
                for g0 in range(0, HC, G):
                    g1 = min(g0 + G, HC)
                    osb = [
                        opool.tile([128, G, 2, NBLK * S * L], F16, name=f"o{ct}_{g0}")
                        for ct in range(2)
                    ]
                    for h in range(g0, g1):
                        for ct in range(2):
                            ps = [
                                pspools[blk].tile(
                                    [128, NCOL], F32, name=f"ps{blk}_{ct}_{h}",
                                    tag=f"psb{blk}",
                                )
                                for blk in range(NBLK)
                            ]
                            for dy in range(KS):
                                for blk in range(NBLK):
                                    lo = KB2 * blk
                                    nc.tensor.matmul(
                                        ps[blk][:, :],
                                        F_sb[lo : lo + KB2, h + dy,
                                             ct * 128 : (ct + 1) * 128],
                                        B_sb[lo : lo + KB2, h, dy, :],
                                        start=(dy == 0),
                                        stop=(dy == KS - 1),
                                    )
                            dstv = osb[ct][:, h - g0].rearrange(
                                "p a (k w) -> p a k w", k=NBLK
                            )
                            for blk in range(NBLK):
                                src = ps[blk][:].rearrange("p (a w) -> p a w", a=2)
                                dst = dstv[:, :, blk, :]
                                if blk == 0:
                                    nc.vector.tensor_copy(dst, src)
                                else:
                                    nc.scalar.copy(dst, src)
                    for ct in range(2):
                        nc.scalar.dma_start(
                            out[ct * 128 : (ct + 1) * 128, S * g0 : S * g1, :],
                            osb[ct][:, : g1 - g0].rearrange("p g a c -> p (g a c)"),
                        )

            if iters == 1:
                body()
            else:
                with tc.For_i(0, iters, 1) as _i:
                    body(_i)
    nc.finalize()
    return nc


host_prep_v5 = host_prep_v2


# ---------------- v7: single-queue FIFO ordering (outs behind the band) ----------------


def build_program_v7(iters: int = 1, dt=F16, psbufs: int = 4, obufs: int = 3,
                     out_group: int = 5, bsplit=(2, 3, 4, 4, 6, 6),
                     fsplit=(9, 10, 10)):
    """v5 compute; all band chunks AND output DMAs ride the sync HWDGE queue
    (FIFO => every out-DMA waits for the full band, so the PE never starves),
    while the small feature stream lands in parallel on the scalar queue."""
    nc = bacc.Bacc(None, target_bir_lowering=False, debug=False)
    f_in = nc.dram_tensor("f", [128, NR, C], dt, kind="ExternalInput")
    b_in = nc.dram_tensor("b", [128, HC, KS, NCOL], dt, kind="ExternalInput")
    out = nc.dram_tensor("out", [C, S * HC, S * W], F16, kind="ExternalOutput")

    with tile.TileContext(nc) as tc:
        with (
            tc.tile_pool(name="fsb", bufs=1) as fpool,
            tc.tile_pool(name="bsb", bufs=1) as bpool,
            tc.tile_pool(name="osb", bufs=obufs) as opool,
            tc.tile_pool(name="ps0", bufs=psbufs, space="PSUM") as pspool0,
            tc.tile_pool(name="ps1", bufs=psbufs, space="PSUM") as pspool1,
        ):
            pspools = [pspool0, pspool1]

            def body(_=None):
                F_sb = fpool.tile([128, NR, C], dt)
                B_sb = bpool.tile([128, HC, KS, NCOL], dt)
                h0 = 0
                for bs in bsplit:
                    nc.sync.dma_start(B_sb[:, h0 : h0 + bs], b_in[:, h0 : h0 + bs])
                    h0 += bs
                r0 = 0
                for fs in fsplit:
                    nc.scalar.dma_start(F_sb[:, r0 : r0 + fs], f_in[:, r0 : r0 + fs])
                    r0 += fs
                G = out_group
                for g0 in range(0, HC, G):
                    g1 = min(g0 + G, HC)
                    osb = [
                        opool.tile([128, G, 2, NBLK * S * L], F16, name=f"o{ct}_{g0}")
                        for ct in range(2)
                    ]
                    for h in range(g0, g1):
                        for ct in range(2):
                            ps = [
                                pspools[blk].tile(
                                    [128, NCOL], F32, name=f"ps{blk}_{ct}_{h}",
                                    tag=f"psb{blk}",
                                )
                                for blk in range(NBLK)
                            ]
                            for dy in range(KS):
                                for blk in range(NBLK):
                                    lo = KB2 * blk
                                    nc.tensor.matmul(
                                        ps[blk][:, :],
                                        F_sb[lo : lo + KB2, h + dy,
                                             ct * 128 : (ct + 1) * 128],
                                        B_sb[lo : lo + KB2, h, dy, :],
                                        start=(dy == 0),
                                        stop=(dy == KS - 1),
                                    )
                            dstv = osb[ct][:, h - g0].rearrange(
                                "p a (k w) -> p a k w", k=NBLK
                            )
                            for blk in range(NBLK):
                                src = ps[blk][:].rearrange("p (a w) -> p a w", a=2)
                                dst = dstv[:, :, blk, :]
                                if blk == 0:
                                    nc.vector.tensor_copy(dst, src)
                                else:
                                    nc.scalar.copy(dst, src)
                    for ct in range(2):
                        nc.sync.dma_start(
                            out[ct * 128 : (ct + 1) * 128, S * g0 : S * g1, :],
                            osb[ct][:, : g1 - g0].rearrange("p g a c -> p (g a c)"),
                        )

            if iters == 1:
                body()
            else:
                with tc.For_i(0, iters, 1) as _i:
                    body(_i)
    nc.finalize()
    return nc


host_prep_v7 = host_prep_v2


# ---------------- v10: hybrid band build (DMA head/tail + GPSIMD scatter middle) ----------------
SC_LO, SC_HI = 2, 14   # rows built by local_scatter: h in [SC_LO, SC_HI)


def build_program_v10(iters: int = 1, dt=F16, psbufs: int = 4, obufs: int = 3,
                      groups=(5, 5, 5, 5, 5), scpair: int = 2):
    """Hybrid band sourcing: rows h0-1 and h14-24 arrive as host-built band
    via DMA (first matmul gated only by the small h0-1 chunk); rows h2-13 are
    built on-device by GPSIMD local_scatter from compact masks (d, 0.3 MB vs
    3.1 MB of band). A dependency-free dummy scatter at program start hoists
    the ~5us GPSIMD library load off the critical path. Output DMAs ride the
    sync queue FIFO behind the (now small) band stream."""
    nsc = SC_HI - SC_LO
    nc = bacc.Bacc(None, target_bir_lowering=False, debug=False)
    f_in = nc.dram_tensor("f", [128, NR, C], dt, kind="ExternalInput")
    b0_in = nc.dram_tensor("b0", [128, SC_LO, KS, NCOL], dt, kind="ExternalInput")
    bt_in = nc.dram_tensor("bt", [128, HC - SC_HI, KS, NCOL], dt,
                           kind="ExternalInput")
    d_in = nc.dram_tensor("d", [128, nsc, NIDX], dt, kind="ExternalInput")
    ix_in = nc.dram_tensor("idx", [128, scpair * NIDX], mybir.dt.int16,
                           kind="ExternalInput")
    out = nc.dram_tensor("out", [C, S * HC, S * W], F16, kind="ExternalOutput")

    with tile.TileContext(nc) as tc:
        with (
            tc.tile_pool(name="fsb", bufs=1) as fpool,
            tc.tile_pool(name="bsb", bufs=1) as bpool,
            tc.tile_pool(name="dsb", bufs=1) as dpool,
            tc.tile_pool(name="osb", bufs=obufs) as opool,
            tc.tile_pool(name="ps0", bufs=psbufs, space="PSUM") as pspool0,
            tc.tile_pool(name="ps1", bufs=psbufs, space="PSUM") as pspool1,
        ):
            pspools = [pspool0, pspool1]

            def body(_=None):
                F_sb = fpool.tile([128, NR, C], dt)
                B_sb = bpool.tile([128, HC, KS, NCOL], dt)
                D_sb = dpool.tile([128, nsc, NIDX], dt)
                I_sb = dpool.tile([128, scpair * NIDX], mybir.dt.int16, name="isb")
                dum = dpool.tile([128, 8], mybir.dt.int16, name="dum")
                # dependency-free dummy scatter: hoists the GPSIMD lib load
                nc.gpsimd.memset(dum[:], -1)
                nc.gpsimd.local_scatter(
                    dum[:, 0:2], dum[:, 2:6], dum[:, 2:6],
                    channels=128, num_elems=2, num_idxs=4,
                )
                # sync queue: first-rows band, idx, compact masks, tail band, outs
                nc.sync.dma_start(B_sb[:, :SC_LO], b0_in[:])
                nc.sync.dma_start(I_sb[:], ix_in[:])
                step = (nsc + scpair * 2 - 1) // (scpair * 2)
                for k in range(0, nsc, 4):
                    k1 = min(k + 4, nsc)
                    nc.sync.dma_start(D_sb[:, k:k1], d_in[:, k:k1])
                nt = HC - SC_HI
                for k0, k1 in ((0, nt // 2), (nt // 2, nt)):
                    nc.sync.dma_start(
                        B_sb[:, SC_HI + k0 : SC_HI + k1], bt_in[:, k0:k1]
                    )
                r0 = 0
                for fs in (9, 10, 10):
                    nc.scalar.dma_start(F_sb[:, r0 : r0 + fs], f_in[:, r0 : r0 + fs])
                    r0 += fs
                # scatter-built middle rows, scpair rows per op
                for h in range(SC_LO, SC_HI, scpair):
                    nc.gpsimd.local_scatter(
                        B_sb[:, h : h + scpair].rearrange("p g d c -> p (g d c)"),
                        D_sb[:, h - SC_LO : h - SC_LO + scpair].rearrange(
                            "p g c -> p (g c)"),
                        I_sb[:],
                        channels=128,
                        num_elems=scpair * KS * NCOL,
                        num_idxs=scpair * NIDX,
                    )
                g0 = 0
                maxg = max(groups)
                for G in groups:
                    g1 = g0 + G
                    osb = [
                        opool.tile([128, maxg, 2, NBLK * S * L], F16,
                                   name=f"o{ct}_{g0}")
                        for ct in range(2)
                    ]
                    for h in range(g0, g1):
                        for ct in range(2):
                            ps = [
                                pspools[blk].tile(
                                    [128, NCOL], F32, name=f"ps{blk}_{ct}_{h}",
                                    tag=f"psb{blk}",
                                )
                                for blk in range(NBLK)
                            ]
                            for dy in range(KS):
                                for blk in range(NBLK):
                                    lo = KB2 * blk
                                    nc.tensor.matmul(
                                        ps[blk][:, :],
                                        F_sb[lo : lo + KB2, h + dy,
                                             ct * 128 : (ct + 1) * 128],
                                        B_sb[lo : lo + KB2, h, dy, :],
                                        start=(dy == 0),
                                        stop=(dy == KS - 1),
                                    )
                            dstv = osb[ct][:, h - g0].rearrange(
                                "p a (k w) -> p a k w", k=NBLK
                            )
                            for blk in range(NBLK):
                                src = ps[blk][:].rearrange("p (a w) -> p a w", a=2)
                                dst = dstv[:, :, blk, :]
                                if blk == 0:
                                    nc.vector.tensor_copy(dst, src)
                                else:
                                    nc.scalar.copy(dst, src)
                    for ct in range(2):
                        nc.sync.dma_start(
                            out[ct * 128 : (ct + 1) * 128, S * g0 : S * g1, :],
                            osb[ct][:, : g1 - g0].rearrange("p g a c -> p (g a c)"),
                        )
                    g0 = g1

            if iters == 1:
                body()
            else:
                with tc.For_i(0, iters, 1) as _i:
                    body(_i)
    nc.finalize()
    return nc


def host_prep_v10(features: np.ndarray, masks: np.ndarray):
    """v10 inputs: f as v2; b0/bt = band rows [0,SC_LO) and [SC_HI,HC);
    d = pre-shifted compact masks for rows [SC_LO,SC_HI); idx = scatter table
    for `scpair` consecutive rows (second row offset by one band slab)."""
    f_hosts, b_hosts = host_prep_v2(features, masks)
    nsc = SC_HI - SC_LO
    wl_idx = np.arange(L)
    aux = []
    for core in range(8):
        n, q = divmod(core, 4)
        h0 = HC * q
        m7 = masks[n].reshape(KS, KS, H, S, W, S)[:, :, h0 : h0 + HC]
        d_host = np.zeros((128, nsc, KS, KS, 2, 2), np.float16)
        for blk in range(NBLK):
            for u in range(KS):
                src = m7[:, u, SC_LO:SC_HI, :, L * blk + wl_idx, :]  # [wl,dy,h,a,b]
                d_host[KB2 * blk + u + wl_idx, :, :, u, :, :] = (
                    src.transpose(0, 2, 1, 3, 4)
                )
        b_full = b_hosts[core].reshape(128, HC, KS, NCOL)
        aux.append({
            "b0": np.ascontiguousarray(b_full[:, :SC_LO]),
            "bt": np.ascontiguousarray(b_full[:, SC_HI:]),
            "d": d_host.reshape(128, nsc, NIDX),
        })
    idx = np.full((128, KS, KS, 2, 2), -1, np.int16)
    for blk in range(NBLK):
        for pp in range(KB):
            for u in range(KS):
                wl = pp - u
                if 0 <= wl < L:
                    for dy in range(KS):
                        for a in range(2):
                            for b in range(2):
                                idx[KB2 * blk + pp, dy, u, a, b] = (
                                    ((dy * 2 + a) * L + wl) * 2 + b
                                )
    idx1 = idx.reshape(128, NIDX)
    idx2 = np.concatenate([idx1, np.where(idx1 >= 0, idx1 + KS * NCOL, -1)],
                         axis=1).astype(np.int16)
    for core in range(8):
        aux[core]["idx"] = idx2
    return f_hosts, aux


# ---------------- v9: v7 retuned (no warmup, fewer chunks, tail options) ----------------


def build_program_v9(iters: int = 1, dt=F16, psbufs: int = 4, obufs: int = 3,
                     groups=(5, 5, 5, 5, 5), bsplit=(2, 3, 4, 8, 8),
                     fsplit=(9, 10, 10), warm: int = 0):
    """v7 with chunk counts tuned for the 11-semaphore DMA pool: 5 band
    chunks + 3 feature chunks issue up-front without semaphore recycling
    stalls; outputs FIFO behind the band on the sync queue."""
    nc = bacc.Bacc(None, target_bir_lowering=False, debug=False)
    f_in = nc.dram_tensor("f", [128, NR, C], dt, kind="ExternalInput")
    b_in = nc.dram_tensor("b", [128, HC, KS, NCOL], dt, kind="ExternalInput")
    out = nc.dram_tensor("out", [C, S * HC, S * W], F16, kind="ExternalOutput")

    with tile.TileContext(nc) as tc:
        with (
            tc.tile_pool(name="fsb", bufs=1) as fpool,
            tc.tile_pool(name="bsb", bufs=1) as bpool,
            tc.tile_pool(name="osb", bufs=obufs) as opool,
            tc.tile_pool(name="ps0", bufs=psbufs, space="PSUM") as pspool0,
            tc.tile_pool(name="ps1", bufs=psbufs, space="PSUM") as pspool1,
        ):
            pspools = [pspool0, pspool1]

            def body(_=None):
                F_sb = fpool.tile([128, NR, C], dt)
                B_sb = bpool.tile([128, HC, KS, NCOL], dt)
                if warm == -1:
                    # interleave: scalar carries F plus every other B chunk
                    r0 = fsplit[0]
                    nc.scalar.dma_start(F_sb[:, :r0], f_in[:, :r0])
                    h0 = 0
                    fi = 1
                    for bi, bs in enumerate(bsplit):
                        eng = nc.scalar if bi % 2 == 1 else nc.sync
                        eng.dma_start(B_sb[:, h0 : h0 + bs], b_in[:, h0 : h0 + bs])
                        h0 += bs
                        if bi % 2 == 1 and fi < len(fsplit):
                            fs = fsplit[fi]
                            nc.scalar.dma_start(
                                F_sb[:, r0 : r0 + fs], f_in[:, r0 : r0 + fs]
                            )
                            r0 += fs
                            fi += 1
                else:
                    h0 = 0
                    for bs in bsplit:
                        nc.sync.dma_start(
                            B_sb[:, h0 : h0 + bs], b_in[:, h0 : h0 + bs]
                        )
                        h0 += bs
                    r0 = 0
                    for fs in fsplit:
                        nc.scalar.dma_start(
                            F_sb[:, r0 : r0 + fs], f_in[:, r0 : r0 + fs]
                        )
                        r0 += fs
                if warm > 0:
                    # keep the PE busy through the DMA lead-in so the HAM
                    # clock gate is at 2.4 GHz when real matmuls start
                    wsb = fpool.tile([128, 128], dt, name="wsb")
                    nc.vector.memset(wsb[:], 0.0)
                    wps = pspool1.tile([128, 128], F32, name="warmps", tag="psb1")
                    for k in range(warm):
                        nc.tensor.matmul(
                            wps[:, :], wsb[:, :], wsb[:, :],
                            start=(k == 0), stop=(k == warm - 1),
                        )
                g0 = 0
                maxg = max(groups)
                for G in groups:
                    g1 = g0 + G
                    osb = [
                        opool.tile([128, maxg, 2, NBLK * S * L], F16,
                                   name=f"o{ct}_{g0}")
                        for ct in range(2)
                    ]
                    for h in range(g0, g1):
                        for ct in range(2):
                            ps = [
                                pspools[blk].tile(
                                    [128, NCOL], F32, name=f"ps{blk}_{ct}_{h}",
                                    tag=f"psb{blk}",
                                )
                                for blk in range(NBLK)
                            ]
                            for dy in range(KS):
                                for blk in range(NBLK):
                                    lo = KB2 * blk
                                    nc.tensor.matmul(
                                        ps[blk][:, :],
                                        F_sb[lo : lo + KB2, h + dy,
                                             ct * 128 : (ct + 1) * 128],
                                        B_sb[lo : lo + KB2, h, dy, :],
                                        start=(dy == 0),
                                        stop=(dy == KS - 1),
                                    )
                            dstv = osb[ct][:, h - g0].rearrange(
                                "p a (k w) -> p a k w", k=NBLK
                            )
                            for blk in range(NBLK):
                                src = ps[blk][:].rearrange("p (a w) -> p a w", a=2)
                                dst = dstv[:, :, blk, :]
                                if blk == 0:
                                    nc.vector.tensor_copy(dst, src)
                                else:
                                    nc.scalar.copy(dst, src)
                    for ct in range(2):
                        nc.sync.dma_start(
                            out[ct * 128 : (ct + 1) * 128, S * g0 : S * g1, :],
                            osb[ct][:, : g1 - g0].rearrange("p g a c -> p (g a c)"),
                        )
                    g0 = g1

            if iters == 1:
                body()
            else:
                with tc.For_i(0, iters, 1) as _i:
                    body(_i)
    nc.finalize()
    return nc


host_prep_v9 = host_prep_v2


# ---------------- v8: v7 + tiny lead chunks + PE warmup + finer out tail ----------------


def build_program_v8(iters: int = 1, dt=F16, psbufs: int = 4, obufs: int = 3,
                     groups=(5, 5, 5, 4, 3, 3), bsplit=(1, 2, 3, 4, 5, 5, 5),
                     fsplit=(5, 8, 8, 8), warm: int = 16):
    """v7 + (a) 1-row first band chunk so the first matmul fires ASAP,
    (b) dummy matmuls on scratch during the DMA lead-in to lift the PE HAM
    clock gate to 2.4 GHz before real work, (c) smaller trailing output
    groups so the final out-DMA after the last matmul is short."""
    nc = bacc.Bacc(None, target_bir_lowering=False, debug=False)
    f_in = nc.dram_tensor("f", [128, NR, C], dt, kind="ExternalInput")
    b_in = nc.dram_tensor("b", [128, HC, KS, NCOL], dt, kind="ExternalInput")
    out = nc.dram_tensor("out", [C, S * HC, S * W], F16, kind="ExternalOutput")

    with tile.TileContext(nc) as tc:
        with (
            tc.tile_pool(name="fsb", bufs=1) as fpool,
            tc.tile_pool(name="bsb", bufs=1) as bpool,
            tc.tile_pool(name="osb", bufs=obufs) as opool,
            tc.tile_pool(name="ps0", bufs=psbufs, space="PSUM") as pspool0,
            tc.tile_pool(name="ps1", bufs=psbufs, space="PSUM") as pspool1,
        ):
            pspools = [pspool0, pspool1]

            def body(_=None):
                F_sb = fpool.tile([128, NR, C], dt)
                B_sb = bpool.tile([128, HC, KS, NCOL], dt)
                if warm:
                    wsb = fpool.tile([128, 512], dt, name="wsb")
                    nc.vector.memset(wsb[:], 0.0)
                    wps = pspool0.tile([128, 512], F32, name="warmps", tag="psb0")
                    for k in range(warm):
                        nc.tensor.matmul(
                            wps[:, :], wsb[:, 0:128], wsb[:, :],
                            start=(k == 0), stop=(k == warm - 1),
                        )
                h0 = 0
                for bs in bsplit:
                    nc.sync.dma_start(B_sb[:, h0 : h0 + bs], b_in[:, h0 : h0 + bs])
                    h0 += bs
                r0 = 0
                for fs in fsplit:
                    nc.scalar.dma_start(F_sb[:, r0 : r0 + fs], f_in[:, r0 : r0 + fs])
                    r0 += fs
                g0 = 0
                for G in groups:
                    g1 = g0 + G
                    osb = [
                        opool.tile([128, groups[0], 2, NBLK * S * L], F16,
                                   name=f"o{ct}_{g0}")
                        for ct in range(2)
                    ]
                    for h in range(g0, g1):
                        for ct in range(2):
                            ps = [
                                pspools[blk].tile(
                                    [128, NCOL], F32, name=f"ps{blk}_{ct}_{h}",
                                    tag=f"psb{blk}",
                                )
                                for blk in range(NBLK)
                            ]
                            for dy in range(KS):
                                for blk in range(NBLK):
                                    lo = KB2 * blk
                                    nc.tensor.matmul(
                                        ps[blk][:, :],
                                        F_sb[lo : lo + KB2, h + dy,
                                             ct * 128 : (ct + 1) * 128],
                                        B_sb[lo : lo + KB2, h, dy, :],
                                        start=(dy == 0),
                                        stop=(dy == KS - 1),
                                    )
                            dstv = osb[ct][:, h - g0].rearrange(
                                "p a (k w) -> p a k w", k=NBLK
                            )
                            for blk in range(NBLK):
                                src = ps[blk][:].rearrange("p (a w) -> p a w", a=2)
                                dst = dstv[:, :, blk, :]
                                if blk == 0:
                                    nc.vector.tensor_copy(dst, src)
                                else:
                                    nc.scalar.copy(dst, src)
                    for ct in range(2):
                        nc.sync.dma_start(
                            out[ct * 128 : (ct + 1) * 128, S * g0 : S * g1, :],
                            osb[ct][:, : g1 - g0].rearrange("p g a c -> p (g a c)"),
                        )
                    g0 = g1

            if iters == 1:
                body()
            else:
                with tc.For_i(0, iters, 1) as _i:
                    body(_i)
    nc.finalize()
    return nc


host_prep_v8 = host_prep_v2


# ---------------- v6: on-device band build via GPSIMD local_scatter ----------------
NIDX = KS * KS * 2 * 2  # 100 scatter slots per partition: (dy, u=dx, a, b)


def build_program_v6(iters: int = 1, dt=F16, psbufs: int = 4, obufs: int = 3,
                     out_group: int = 5, dsplit=(5, 5, 5, 5, 5), fsplit=(9, 10, 10)):
    """v5 compute, but the banded mask matrices are built ON DEVICE:
    HBM carries only the compact masks pre-shifted per dx (d: 0.64 MB vs the
    6.4 MB host-built band) plus a 25.6 KB constant index table. One GPSIMD
    local_scatter per low-res row h places the 100 per-partition mask values
    onto the band diagonals (and zeroes the rest of that row's band slab)."""
    nc = bacc.Bacc(None, target_bir_lowering=False, debug=False)
    f_in = nc.dram_tensor("f", [128, NR, C], dt, kind="ExternalInput")
    d_in = nc.dram_tensor("d", [128, HC, NIDX], dt, kind="ExternalInput")
    ix_in = nc.dram_tensor("idx", [128, NIDX], mybir.dt.int16, kind="ExternalInput")
    out = nc.dram_tensor("out", [C, S * HC, S * W], F16, kind="ExternalOutput")

    with tile.TileContext(nc) as tc:
        with (
            tc.tile_pool(name="fsb", bufs=1) as fpool,
            tc.tile_pool(name="bsb", bufs=1) as bpool,
            tc.tile_pool(name="dsb", bufs=1) as dpool,
            tc.tile_pool(name="osb", bufs=obufs) as opool,
            tc.tile_pool(name="ps0", bufs=psbufs, space="PSUM") as pspool0,
            tc.tile_pool(name="ps1", bufs=psbufs, space="PSUM") as pspool1,
        ):
            pspools = [pspool0, pspool1]

            def body(_=None):
                F_sb = fpool.tile([128, NR, C], dt)
                B_sb = bpool.tile([128, HC, KS, NCOL], dt)
                D_sb = dpool.tile([128, HC, NIDX], dt)
                I_sb = dpool.tile([128, NIDX], mybir.dt.int16, name="isb")
                nc.sync.dma_start(I_sb[:], ix_in[:])
                h0 = 0
                for ds in dsplit:
                    nc.sync.dma_start(D_sb[:, h0 : h0 + ds], d_in[:, h0 : h0 + ds])
                    h0 += ds
                r0 = 0
                for fs in fsplit:
                    nc.scalar.dma_start(F_sb[:, r0 : r0 + fs], f_in[:, r0 : r0 + fs])
                    r0 += fs
                G = out_group
                for g0 in range(0, HC, G):
                    g1 = min(g0 + G, HC)
                    osb = [
                        opool.tile([128, G, 2, NBLK * S * L], F16, name=f"o{ct}_{g0}")
                        for ct in range(2)
                    ]
                    for h in range(g0, g1):
                        nc.gpsimd.local_scatter(
                            B_sb[:, h].rearrange("p d c -> p (d c)"),
                            D_sb[:, h],
                            I_sb[:],
                            channels=128,
                            num_elems=KS * NCOL,
                            num_idxs=NIDX,
                        )
                        for ct in range(2):
                            ps = [
                                pspools[blk].tile(
                                    [128, NCOL], F32, name=f"ps{blk}_{ct}_{h}",
                                    tag=f"psb{blk}",
                                )
                                for blk in range(NBLK)
                            ]
                            for dy in range(KS):
                                for blk in range(NBLK):
                                    lo = KB2 * blk
                                    nc.tensor.matmul(
                                        ps[blk][:, :],
                                        F_sb[lo : lo + KB2, h + dy,
                                             ct * 128 : (ct + 1) * 128],
                                        B_sb[lo : lo + KB2, h, dy, :],
                                        start=(dy == 0),
                                        stop=(dy == KS - 1),
                                    )
                            dstv = osb[ct][:, h - g0].rearrange(
                                "p a (k w) -> p a k w", k=NBLK
                            )
                            for blk in range(NBLK):
                                src = ps[blk][:].rearrange("p (a w) -> p a w", a=2)
                                dst = dstv[:, :, blk, :]
                                if blk == 0:
                                    nc.vector.tensor_copy(dst, src)
                                else:
                                    nc.scalar.copy(dst, src)
                    for ct in range(2):
                        nc.scalar.dma_start(
                            out[ct * 128 : (ct + 1) * 128, S * g0 : S * g1, :],
                            osb[ct][:, : g1 - g0].rearrange("p g a c -> p (g a c)"),
                        )

            if iters == 1:
                body()
            else:
                with tc.For_i(0, iters, 1) as _i:
                    body(_i)
    nc.finalize()
    return nc


def host_prep_v6(features: np.ndarray, masks: np.ndarray):
    """v6 inputs: F as in v2; d = per-dx partition-shifted compact masks
    [128, HC, (dy, u, a, b)]; idx = constant int16 scatter table [128, 100]."""
    f_hosts, _ = host_prep_v2(features, masks)
    d_hosts = []
    wl_idx = np.arange(L)
    for core in range(8):
        n, q = divmod(core, 4)
        h0 = HC * q
        m7 = masks[n].reshape(KS, KS, H, S, W, S)[:, :, h0 : h0 + HC]
        d_host = np.zeros((128, HC, KS, KS, 2, 2), np.float16)
        for blk in range(NBLK):
            for u in range(KS):
                src = m7[:, u, :, :, L * blk + wl_idx, :]  # [wl, dy, h, a, b]
                d_host[KB2 * blk + u + wl_idx, :, :, u, :, :] = (
                    src.transpose(0, 2, 1, 3, 4)
                )
        d_hosts.append(d_host.reshape(128, HC, NIDX))
    # scatter index table: for band partition q = 64*blk + p', slot (dy,u,a,b)
    # lands at free index ((dy*2 + a)*L + (p'-u))*2 + b of the (dy,a,wl,b) row,
    # or -1 (skipped) when wl = p'-u falls outside [0, L).
    idx = np.full((128, KS, KS, 2, 2), -1, np.int16)
    for blk in range(NBLK):
        for pp in range(KB):
            for u in range(KS):
                wl = pp - u
                if 0 <= wl < L:
                    for dy in range(KS):
                        for a in range(2):
                            for b in range(2):
                                idx[KB2 * blk + pp, dy, u, a, b] = (
                                    ((dy * 2 + a) * L + wl) * 2 + b
                                )
    idx_host = idx.reshape(128, NIDX)
    return f_hosts, [
        {"d": d_hosts[c], "idx": idx_host} for c in range(8)
    ]


# ---------------- v3: dy-pairs stacked in K (two taps per matmul) ----------------
NP3 = (KS + 1) // 2  # 3 matmuls per (h, blk): dy pairs (0,1), (2,3), (4,-)


def build_program_v3(iters: int = 1, dt=F16, copy_eng="both", parts="full",
                     psbufs: int = 3, obufs: int = 2, out_group: int = 5,
                     bchunks: int = 5, unroll: bool = False):
    """v3: K=128 = (dy-pair half j in {0,1}) x (w'' in [0,64)). The upper 64
    partitions hold a one-row-shifted copy of the features, so one matmul
    contracts two vertical taps. 300 matmuls of N=200, all tile_position
    (0,0), one PSUM bank per output row."""
    nc = bacc.Bacc(None, target_bir_lowering=False, debug=False)
    f_in = nc.dram_tensor("f", [128, NBLK, NR, C], dt, kind="ExternalInput")
    b_in = nc.dram_tensor("b", [128, NBLK, HC, NP3, NCOL], dt, kind="ExternalInput")
    out = nc.dram_tensor("out", [C, S * HC, S * W], F32, kind="ExternalOutput")

    with tile.TileContext(nc) as tc:
        with (
            tc.tile_pool(name="fsb", bufs=1) as fpool,
            tc.tile_pool(name="bsb", bufs=1) as bpool,
            tc.tile_pool(name="osb", bufs=obufs) as opool,
            tc.tile_pool(name="ps", bufs=psbufs, space="PSUM") as pspool,
        ):
            def body(_=None):
                F_sb = fpool.tile([128, NBLK, NR, C], dt)
                B_sb = bpool.tile([128, NBLK, HC, NP3, NCOL], dt)
                if parts != "nodmain":
                    nc.sync.dma_start(F_sb[:, :, : NR // 2], f_in[:, :, : NR // 2])
                    nc.sync.dma_start(F_sb[:, :, NR // 2 :], f_in[:, :, NR // 2 :])
                    bstep = (HC + bchunks - 1) // bchunks
                    for h0 in range(0, HC, bstep):
                        h1 = min(h0 + bstep, HC)
                        nc.sync.dma_start(B_sb[:, :, h0:h1], b_in[:, :, h0:h1])
                if parts == "dmain":
                    return
                G = out_group
                for ct in range(2):
                    for g0 in range(0, HC, G):
                        g1 = min(g0 + G, HC)
                        osb = opool.tile([128, G, 2, NBLK * S * L], F32)
                        for h in range(g0, g1):
                            ps = pspool.tile(
                                [128, NBLK * NCOL], F32, name=f"ps_{ct}_{h}",
                                tag="ps",
                            )
                            for blk in range(NBLK):
                                for p in range(NP3):
                                    nc.tensor.matmul(
                                        ps[:, blk * NCOL : (blk + 1) * NCOL],
                                        F_sb[:, blk, h + 2 * p,
                                             ct * 128 : (ct + 1) * 128],
                                        B_sb[:, blk, h, p, :],
                                        start=(blk == 0 and p == 0),
                                        stop=(blk == NBLK - 1 and p == NP3 - 1),
                                    )
                            if parts == "nocopy":
                                continue
                            # psum free layout (blk, a, w2l) -> dest (a, blk, w2l)
                            src = ps[:].rearrange("p (k a w) -> p a k w", k=NBLK, a=2)
                            dst = osb[:, h - g0].rearrange(
                                "p a (k w) -> p a k w", k=NBLK
                            )
                            if copy_eng == "vector" or (
                                copy_eng == "both" and h % 2 == 0
                            ):
                                nc.vector.tensor_copy(dst, src)
                            else:
                                nc.scalar.copy(dst, src)
                        if parts == "nocopy":
                            continue
                        nc.sync.dma_start(
                            out[ct * 128 : (ct + 1) * 128, S * g0 : S * g1, :],
                            osb[:, : g1 - g0].rearrange("p g a c -> p (g a c)"),
                        )

            if iters == 1:
                body()
            elif unroll:
                for _k in range(iters):
                    body(_k)
            else:
                with tc.For_i(0, iters, 1) as _i:
                    body(_i)
    nc.finalize()
    return nc


def host_prep_v3(features: np.ndarray, masks: np.ndarray):
    """v3 layouts: partition = 64*j + w''; j=1 half holds features shifted one
    row down (dy-pair trick). Separate windows per width block."""
    f_hosts, b_hosts = [], []
    padded = np.pad(features, ((0, 0), (0, 0), (R, R), (R, R)))
    wl_idx = np.arange(L)
    for core in range(8):
        n, q = divmod(core, 4)
        h0 = HC * q
        F_core = padded[n, :, h0 : h0 + NR, :]  # [C, 29, 104]
        F_host = np.zeros((128, NBLK, NR, C), np.float16)
        for blk in range(NBLK):
            win = F_core[:, :, L * blk : L * blk + KB].transpose(2, 1, 0)  # [54,29,C]
            F_host[:KB, blk] = win                      # j=0: rows r
            F_host[64 : 64 + KB, blk, : NR - 1] = win[:, 1:]  # j=1: rows r+1
        m7 = masks[n].reshape(KS, KS, H, S, W, S)[:, :, h0 : h0 + HC]
        B_host = np.zeros((128, NBLK, HC, NP3, 2, L, 2), np.float16)
        for blk in range(NBLK):
            for dx in range(KS):
                for dy in range(KS):
                    p, j = divmod(dy, 2)
                    src = m7[dy, dx, :, :, L * blk : L * blk + L, :]  # [h,a,wl,b]
                    B_host[64 * j + dx + wl_idx, blk, :, p, :, wl_idx, :] = (
                        src.transpose(2, 0, 1, 3)
                    )
        f_hosts.append(F_host)
        b_hosts.append(B_host.reshape(128, NBLK, HC, NP3, NCOL))
    return f_hosts, b_hosts


_NC_CACHE = {}

# Best HW-verified configuration (single-shot device exec ~51 us/core mean,
# ~56 us max across the 8 cores, vs ~76/82 us for the v2 baseline measured
# the same way):
#   - v2's banded-matmul compute (K=64, two concurrent PE row-group chains,
#     N=200 per matmul, PSUM-accumulated over the 5 vertical taps),
#   - fp16 output staging (halves the out-DMA to 5.12 MB/core; host upcasts),
#   - h-outer/ct-inner loop so each band chunk feeds 2x the matmuls,
#   - band chunks consumption-ordered on the sync HWDGE queue with a tiny
#     first chunk (first matmul fires ~10.5 us in), features on the scalar
#     queue, and all output DMAs FIFO'd on the sync queue BEHIND the band so
#     they can never starve the PE of mask data,
#   - finer trailing output groups to shorten the post-compute DMA tail.
_BEST_KW = dict(groups=(5, 5, 5, 5, 3, 2), bsplit=(1, 2, 4, 4, 7, 7),
                fsplit=(5, 9, 15))


def _get_program(iters: int = 1):
    if iters not in _NC_CACHE:
        _NC_CACHE[iters] = build_program_v9(iters, **_BEST_KW)
    return _NC_CACHE[iters]


def kernel(features: np.ndarray, masks: np.ndarray) -> np.ndarray:
    features = np.ascontiguousarray(features, dtype=np.float32)
    masks = np.ascontiguousarray(masks, dtype=np.float32)
    f_hosts, b_hosts = host_prep_v2(features, masks)
    in_maps = [{"f": f_hosts[c], "b": b_hosts[c]} for c in range(8)]
    nc = _get_program(1)
    res = run_bass_kernel_spmd(nc, in_maps, list(range(8)))
    out = np.empty((N, C, S * H, S * W), np.float32)
    for core in range(8):
        n, q = divmod(core, 4)
        out[n, :, S * HC * q : S * HC * (q + 1), :] = (
            res.results[core]["out"].astype(np.float32)
        )
    return out



# revision 18
# speedup vs baseline: 1.3296x; 1.0297x over previous
"""CARAFE-naive 2x content-aware upsampling on 8 Trainium2 NeuronCores.

Problem: features [2, 256, 100, 100] f32, masks [2, 25, 200, 200] f32
-> out [2, 256, 200, 200] f32, where each output pixel is a 25-tap (5x5)
weighted sum of the source neighborhood, weights shared across channels.

Strategy (per core = one (image n, row-quarter q) pair):
  The 25-tap contraction is cast as TensorE matmuls via a banded-matrix
  trick along the width axis. For one low-res output row h and width
  block of L=50 low-res columns, the contraction over the 5 horizontal
  taps is a matmul with contraction dim K = L+4 = 54 (the padded width
  window): out[c, (a, w2)] = sum_w' F[w', c] * Band[w', (a, w2)], where
  Band packs mask values on 5 diagonals (built host-side in numpy).
  The 5 vertical taps (dy) accumulate in PSUM across 5 matmuls.

  lhsT = transposed feature row slices (stationary), rhs = banded mask
  blocks. Both fp16 (PE runs fp16 at full rate; ~2^-11 rel precision).
  The two width blocks sit at SBUF partition bases 0/64 and run as two
  concurrent PE row-group chains, each accumulating into its own PSUM
  bank (build_program_v9, the shipping configuration).

Single-shot latency decisions (see _BEST_KW): fp16 output staging
(host upcasts to f32), h-outer/ct-inner loop order, band chunks
consumption-ordered on the sync HWDGE queue with output DMAs FIFO'd
behind them, features on the scalar queue, finer trailing out groups.

Host-side numpy does layout/packing only (transpose, pad, diagonal
scatter of masks into band matrices); all FLOPs run on the device.
"""

import numpy as np

import concourse.mybir as mybir
import concourse.tile as tile
from concourse import bacc
from concourse.bass_utils import run_bass_kernel_spmd

# problem constants
N, C, H, W = 2, 256, 100, 100
KS = 5        # kernel size
S = 2         # upsample scale
R = (KS - 1) // 2

# sharding / blocking constants
HC = H // 4       # 25 low-res rows per core (8 cores = 2 images x 4 quarters)
NR = HC + 2 * R   # 29 padded feature rows per core
NBLK = 2          # width blocks
L = W // NBLK     # 50 low-res columns per block
KB = L + KS - 1   # 54 = matmul contraction size
PBASE = 64        # SBUF partition base stride between blocks
NCOL = 2 * S * L  # 200 matmul N per block: (a in 2, w2l in 100)
F16 = mybir.dt.float16
F32 = mybir.dt.float32


def build_program(iters: int = 1, dt=F16, blks=(0, 1), copy_eng="both", parts="full",
                  in_chunks: int = 1, in_engines=("sync",)):
    """Build the per-core bass program. `iters`>1 wraps the whole compute in
    a hardware loop (used only for benchmarking slope timing)."""
    nc = bacc.Bacc(None, target_bir_lowering=False, debug=False)
    f_in = nc.dram_tensor("f", [KB, NBLK, NR, C], dt, kind="ExternalInput")
    b_in = nc.dram_tensor("b", [KB, NBLK, HC, KS, NCOL], dt, kind="ExternalInput")
    out = nc.dram_tensor("out", [C, S * HC, S * W], F32, kind="ExternalOutput")

    with tile.TileContext(nc) as tc:
        with (
            tc.tile_pool(name="fsb", bufs=1) as fpool,
            tc.tile_pool(name="bsb", bufs=1) as bpool,
            tc.tile_pool(name="osb", bufs=4) as opool,
            tc.tile_pool(name="ps", bufs=6, space="PSUM") as pspool,
        ):
            def body(_=None):
                F_sb = fpool.tile([KB, NBLK, NR, C], dt)
                B_sb = bpool.tile([KB, NBLK, HC, KS, NCOL], dt)
                if parts == "dmain128":
                    # DMA-bandwidth probe: same bytes, 108-partition layout
                    F2 = fpool.tile([KB * NBLK, NR, C], dt, name="F2")
                    B2 = bpool.tile([KB * NBLK, HC, KS, NCOL], dt, name="B2")
                    f2 = f_in[:].rearrange("k n r c -> (k n) r c")
                    b2 = b_in[:].rearrange("k n h d c -> (k n) h d c")
                    engs = [getattr(nc, e) for e in in_engines]
                    step = (KB * NBLK + in_chunks - 1) // in_chunks
                    for i, p0 in enumerate(range(0, KB * NBLK, step)):
                        p1 = min(p0 + step, KB * NBLK)
                        engs[i % len(engs)].dma_start(F2[p0:p1], f2[p0:p1])
                        engs[i % len(engs)].dma_start(B2[p0:p1], b2[p0:p1])
                    return
                if parts != "nodmain":
                    engs = [getattr(nc, e) for e in in_engines]
                    ei = 0
                    # split each input DMA into in_chunks along a free dim to
                    # engage more DMA queues in parallel
                    fstep = (NR + in_chunks - 1) // in_chunks
                    for r0 in range(0, NR, fstep):
                        r1 = min(r0 + fstep, NR)
                        engs[ei % len(engs)].dma_start(
                            F_sb[:, :, r0:r1], f_in[:, :, r0:r1]
                        )
                        ei += 1
                    bstep = (HC + in_chunks - 1) // in_chunks
                    for h0 in range(0, HC, bstep):
                        h1 = min(h0 + bstep, HC)
                        engs[ei % len(engs)].dma_start(
                            B_sb[:, :, h0:h1], b_in[:, :, h0:h1]
                        )
                        ei += 1
                if parts == "dmain":
                    return
                for ct in range(2):
                    psums = {}
                    for r in range(NR):
                        for blk in blks:
                            lhsT = F_sb[:, blk, r, ct * 128 : (ct + 1) * 128]
                            for dy in range(KS):
                                h = r - dy
                                if not (0 <= h < HC):
                                    continue
                                if dy == 0 and blk == blks[0]:
                                    psums[h] = pspool.tile(
                                        [128, NBLK * NCOL],
                                        F32,
                                        name=f"ps{ct}_{h}",
                                        tag="ps",
                                    )
                                # One accumulation group per PSUM bank: start
                                # zeroes the whole 2KB zero-region, so only
                                # the first matmul of the tile starts and only
                                # the last one stops.
                                nc.tensor.matmul(
                                    psums[h][:, blk * NCOL : (blk + 1) * NCOL],
                                    lhsT,
                                    B_sb[:, blk, h, dy, :],
                                    start=(dy == 0 and blk == blks[0]),
                                    stop=(dy == KS - 1 and blk == blks[-1]),
                                )
                        h_done = r - (KS - 1)
                        if h_done >= 0 and parts in ("full", "nodmain"):
                            ps = psums.pop(h_done)
                            osb = opool.tile([128, 2, NBLK, S * L], F32)
                            # psum free layout (blk, a, w2l) -> (a, blk, w2l)
                            src = ps[:].rearrange(
                                "p (k a w) -> p a k w", k=NBLK, a=2
                            )
                            if copy_eng == "vector" or (copy_eng == "both" and h_done % 2 == 0):
                                nc.vector.tensor_copy(osb[:], src)
                            else:
                                nc.scalar.copy(osb[:], src)
                            nc.sync.dma_start(
                                out[ct * 128 : (ct + 1) * 128,
                                    S * h_done : S * h_done + 2, :],
                                osb[:].rearrange("p a k w -> p a (k w)"),
                            )

            if iters == 1:
                body()
            else:
                with tc.For_i(0, iters, 1) as _i:
                    body(_i)
    nc.finalize()
    return nc


def host_prep(features: np.ndarray, masks: np.ndarray):
    """Pack per-core fp16 inputs: transposed padded feature rows and banded
    mask matrices. Pure layout work (no arithmetic beyond dtype cast)."""
    f_hosts, b_hosts = [], []
    padded = np.pad(features, ((0, 0), (0, 0), (R, R), (R, R)))  # [N,C,H+4,W+4]
    wl_idx = np.arange(L)
    for core in range(8):
        n, q = divmod(core, 4)
        h0 = HC * q
        F_core = padded[n, :, h0 : h0 + NR, :]  # [C, 29, 104]
        F_host = np.zeros((KB, NBLK, NR, C), np.float16)
        for blk in range(NBLK):
            F_host[:, blk] = F_core[:, :, L * blk : L * blk + KB].transpose(2, 1, 0)
        # masks[n]: [25, 200, 200] -> [dy, dx, h, a, w, b]
        m7 = masks[n].reshape(KS, KS, H, S, W, S)[:, :, h0 : h0 + HC]
        B_host = np.zeros((KB, NBLK, HC, KS, 2, L, 2), np.float16)
        for blk in range(NBLK):
            for dx in range(KS):
                src = m7[:, dx, :, :, L * blk : L * blk + L, :]  # [dy,h,a,wl,b]
                B_host[dx + wl_idx, blk, :, :, :, wl_idx, :] = (
                    src.transpose(3, 1, 0, 2, 4)
                )
        f_hosts.append(F_host)
        b_hosts.append(B_host.reshape(KB, NBLK, HC, KS, NCOL))
    return f_hosts, b_hosts


# ---------------- v2: 128-partition layout, per-block PSUM banks ----------------
KB2 = 64  # padded contraction size (54 useful + 10 zero rows) -> blocks at 0/64


def build_program_v2(iters: int = 1, dt=F16, copy_eng="both", parts="full",
                     psbufs: int = 3, obufs: int = 2, out_group: int = 5,
                     bchunks: int = 5):
    """v2: both width blocks packed on 128 partitions (bases 0/64), each block
    accumulating into its own PSUM bank (documented-safe row-tiling pattern).
    dy-inner loop: weights reload per matmul but the two block chains run
    concurrently on different PE row groups."""
    nc = bacc.Bacc(None, target_bir_lowering=False, debug=False)
    f_in = nc.dram_tensor("f", [128, NR, C], dt, kind="ExternalInput")
    b_in = nc.dram_tensor("b", [128, HC, KS, NCOL], dt, kind="ExternalInput")
    out = nc.dram_tensor("out", [C, S * HC, S * W], F32, kind="ExternalOutput")

    with tile.TileContext(nc) as tc:
        with (
            tc.tile_pool(name="fsb", bufs=1) as fpool,
            tc.tile_pool(name="bsb", bufs=1) as bpool,
            tc.tile_pool(name="osb", bufs=obufs) as opool,
            tc.tile_pool(name="ps0", bufs=psbufs, space="PSUM") as pspool0,
            tc.tile_pool(name="ps1", bufs=psbufs, space="PSUM") as pspool1,
        ):
            pspools = [pspool0, pspool1]

            def body(_=None):
                F_sb = fpool.tile([128, NR, C], dt)
                B_sb = bpool.tile([128, HC, KS, NCOL], dt)
                if parts != "nodmain":
                    # chunked input DMAs: lets matmuls start after chunk 0
                    nc.sync.dma_start(F_sb[:, : NR // 2], f_in[:, : NR // 2])
                    nc.sync.dma_start(F_sb[:, NR // 2 :], f_in[:, NR // 2 :])
                    bstep = (HC + bchunks - 1) // bchunks
                    for h0 in range(0, HC, bstep):
                        h1 = min(h0 + bstep, HC)
                        nc.sync.dma_start(B_sb[:, h0:h1], b_in[:, h0:h1])
                if parts == "dmain":
                    return
                G = out_group
                for ct in range(2):
                    for g0 in range(0, HC, G):
                        g1 = min(g0 + G, HC)
                        osb = opool.tile([128, G, 2, NBLK * S * L], F32)
                        for h in range(g0, g1):
                            ps = [
                                pspools[blk].tile(
                                    [128, NCOL], F32, name=f"ps{blk}_{ct}_{h}",
                                    tag=f"psb{blk}",
                                )
                                for blk in range(NBLK)
                            ]
                            for dy in range(KS):
                                for blk in range(NBLK):
                                    lo = KB2 * blk
                                    nc.tensor.matmul(
                                        ps[blk][:, :],
                                        F_sb[lo : lo + KB2, h + dy,
                                             ct * 128 : (ct + 1) * 128],
                                        B_sb[lo : lo + KB2, h, dy, :],
                                        start=(dy == 0),
                                        stop=(dy == KS - 1),
                                    )
                            if parts == "nocopy":
                                continue
                            # osb free layout per h: (a, blk, w2l) built from the
                            # two psum tiles; dest dims [2, (blk, 100)]
                            dstv = osb[:, h - g0].rearrange(
                                "p a (k w) -> p a k w", k=NBLK
                            )
                            for blk in range(NBLK):
                                src = ps[blk][:].rearrange("p (a w) -> p a w", a=2)
                                dst = dstv[:, :, blk, :]
                                if copy_eng == "vector" or (
                                    copy_eng == "both" and blk == 0
                                ):
                                    nc.vector.tensor_copy(dst, src)
                                else:
                                    nc.scalar.copy(dst, src)
                        if parts == "nocopy":
                            continue
                        nc.sync.dma_start(
                            out[ct * 128 : (ct + 1) * 128,
                                S * g0 : S * g1, :],
                            osb[:, : g1 - g0].rearrange("p g a c -> p (g a c)"),
                        )

            if iters == 1:
                body()
            else:
                with tc.For_i(0, iters, 1) as _i:
                    body(_i)
    nc.finalize()
    return nc


def host_prep_v2(features: np.ndarray, masks: np.ndarray):
    """v2 layouts: [128, ...] with partition = 64*blk + w'' (w'' in [0,54))."""
    f_hosts, b_hosts = [], []
    padded = np.pad(features, ((0, 0), (0, 0), (R, R), (R, R)))
    wl_idx = np.arange(L)
    for core in range(8):
        n, q = divmod(core, 4)
        h0 = HC * q
        F_core = padded[n, :, h0 : h0 + NR, :]  # [C, 29, 104]
        F_host = np.zeros((128, NR, C), np.float16)
        for blk in range(NBLK):
            F_host[KB2 * blk : KB2 * blk + KB] = (
                F_core[:, :, L * blk : L * blk + KB].transpose(2, 1, 0)
            )
        m7 = masks[n].reshape(KS, KS, H, S, W, S)[:, :, h0 : h0 + HC]
        B_host = np.zeros((128, HC, KS, 2, L, 2), np.float16)
        for blk in range(NBLK):
            for dx in range(KS):
                src = m7[:, dx, :, :, L * blk : L * blk + L, :]  # [dy,h,a,wl,b]
                B_host[KB2 * blk + dx + wl_idx, :, :, :, wl_idx, :] = (
                    src.transpose(3, 1, 0, 2, 4)
                )
        f_hosts.append(F_host)
        b_hosts.append(B_host.reshape(128, HC, KS, NCOL))
    return f_hosts, b_hosts


# ---------------- v4: v2 + fp16 output (halves the out-DMA bytes) ----------------


def build_program_v4(iters: int = 1, dt=F16, parts="full",
                     psbufs: int = 3, obufs: int = 2, out_group: int = 5,
                     bchunks: int = 5):
    """v2 compute structure, but the PSUM->SBUF copy downcasts to fp16 and the
    output DRAM tensor is fp16 (host upcasts). Out-DMA drops 10.24 -> 5.12 MB."""
    nc = bacc.Bacc(None, target_bir_lowering=False, debug=False)
    f_in = nc.dram_tensor("f", [128, NR, C], dt, kind="ExternalInput")
    b_in = nc.dram_tensor("b", [128, HC, KS, NCOL], dt, kind="ExternalInput")
    out = nc.dram_tensor("out", [C, S * HC, S * W], F16, kind="ExternalOutput")

    with tile.TileContext(nc) as tc:
        with (
            tc.tile_pool(name="fsb", bufs=1) as fpool,
            tc.tile_pool(name="bsb", bufs=1) as bpool,
            tc.tile_pool(name="osb", bufs=obufs) as opool,
            tc.tile_pool(name="ps0", bufs=psbufs, space="PSUM") as pspool0,
            tc.tile_pool(name="ps1", bufs=psbufs, space="PSUM") as pspool1,
        ):
            pspools = [pspool0, pspool1]

            def body(_=None):
                F_sb = fpool.tile([128, NR, C], dt)
                B_sb = bpool.tile([128, HC, KS, NCOL], dt)
                if parts != "nodmain":
                    nc.sync.dma_start(F_sb[:, : NR // 2], f_in[:, : NR // 2])
                    nc.sync.dma_start(F_sb[:, NR // 2 :], f_in[:, NR // 2 :])
                    bstep = (HC + bchunks - 1) // bchunks
                    for h0 in range(0, HC, bstep):
                        h1 = min(h0 + bstep, HC)
                        nc.sync.dma_start(B_sb[:, h0:h1], b_in[:, h0:h1])
                if parts == "dmain":
                    return
                G = out_group
                for ct in range(2):
                    for g0 in range(0, HC, G):
                        g1 = min(g0 + G, HC)
                        osb = opool.tile([128, G, 2, NBLK * S * L], F16)
                        for h in range(g0, g1):
                            ps = [
                                pspools[blk].tile(
                                    [128, NCOL], F32, name=f"ps{blk}_{ct}_{h}",
                                    tag=f"psb{blk}",
                                )
                                for blk in range(NBLK)
                            ]
                            for dy in range(KS):
                                for blk in range(NBLK):
                                    lo = KB2 * blk
                                    nc.tensor.matmul(
                                        ps[blk][:, :],
                                        F_sb[lo : lo + KB2, h + dy,
                                             ct * 128 : (ct + 1) * 128],
                                        B_sb[lo : lo + KB2, h, dy, :],
                                        start=(dy == 0),
                                        stop=(dy == KS - 1),
                                    )
                            dstv = osb[:, h - g0].rearrange(
                                "p a (k w) -> p a k w", k=NBLK
                            )
                            for blk in range(NBLK):
                                src = ps[blk][:].rearrange("p (a w) -> p a w", a=2)
                                dst = dstv[:, :, blk, :]
                                if blk == 0:
                                    nc.vector.tensor_copy(dst, src)
                                else:
                                    nc.scalar.copy(dst, src)
                        nc.sync.dma_start(
                            out[ct * 128 : (ct + 1) * 128,
                                S * g0 : S * g1, :],
                            osb[:, : g1 - g0].rearrange("p g a c -> p (g a c)"),
                        )

            if iters == 1:
                body()
            else:
                with tc.For_i(0, iters, 1) as _i:
                    body(_i)
    nc.finalize()
    return nc


host_prep_v4 = host_prep_v2


# ---------------- v5: v4 + DMA orchestration for single-shot latency ----------------


def build_program_v5(iters: int = 1, dt=F16, psbufs: int = 4, obufs: int = 3,
                     out_group: int = 5,
                     bsplit=(3, 3, 4, 4, 5, 6), fsplit=(9, 10, 10)):
    """v2 compute + fp16 out + latency-ordered DMAs.

    - h-outer / ct-inner loop: each mask chunk feeds 2x the matmuls, so the
      mask stream only needs half the bandwidth to stay ahead of the PE.
    - b chunks land on the sync HWDGE queue in consumption order (small
      first chunk -> first matmul starts early); f chunks and all output
      DMAs ride the scalar HWDGE queue so outputs never queue behind masks.
    """
    nc = bacc.Bacc(None, target_bir_lowering=False, debug=False)
    f_in = nc.dram_tensor("f", [128, NR, C], dt, kind="ExternalInput")
    b_in = nc.dram_tensor("b", [128, HC, KS, NCOL], dt, kind="ExternalInput")
    out = nc.dram_tensor("out", [C, S * HC, S * W], F16, kind="ExternalOutput")

    with tile.TileContext(nc) as tc:
        with (
            tc.tile_pool(name="fsb", bufs=1) as fpool,
            tc.tile_pool(name="bsb", bufs=1) as bpool,
            tc.tile_pool(name="osb", bufs=obufs) as opool,
            tc.tile_pool(name="ps0", bufs=psbufs, space="PSUM") as pspool0,
            tc.tile_pool(name="ps1", bufs=psbufs, space="PSUM") as pspool1,
        ):
            pspools = [pspool0, pspool1]

            def body(_=None):
                F_sb = fpool.tile([128, NR, C], dt)
                B_sb = bpool.tile([128, HC, KS, NCOL], dt)
                # interleave issue order: B0, F0 first (gates the first MM)
                h0 = 0
                nc.sync.dma_start(B_sb[:, : bsplit[0]], b_in[:, : bsplit[0]])
                r0 = 0
                nc.scalar.dma_start(
                    F_sb[:, : fsplit[0]], f_in[:, : fsplit[0]]
                )
                h0 += bsplit[0]
                for bs in bsplit[1:]:
                    nc.sync.dma_start(B_sb[:, h0 : h0 + bs], b_in[:, h0 : h0 + bs])
                    h0 += bs
                r0 += fsplit[0]
                for fs in fsplit[1:]:
                    nc.scalar.dma_start(
                        F_sb[:, r0 : r0 + fs], f_in[:, r0 : r0 + fs]
                    )
                    r0 += fs
                G = out_group
                for g0 in range(0, HC, G):
                    g1 = min(g0 + G, HC)
                    osb = [
                        opool.tile([128, G, 2, NBLK * S * L], F16, name=f"o{ct}_{g0}")
                        for ct in range(2)
                    ]
                    for h in range(g0, g1):
                        for ct in range(2):
                            ps = [
                                pspools[blk].tile(
                                    [128, NCOL], F32, name=f"ps{blk}_{ct}_{h}",
                                    tag=f"psb{blk}",
                                )
                                for blk in range(NBLK)
                            ]
                            for dy in range(KS):
                                for blk in range(NBLK):
                                    lo = KB2 * blk
                                    nc.tensor.matmul(
                                        ps[blk][:, :],
                                        F_sb[lo : lo + KB2, h + dy,
                                             ct * 128 : (ct + 1) * 128],
                                        B_sb[lo : lo + KB2, h, dy, :],
                                        start=(dy == 0),
                                        stop=(dy == KS - 1),
                                    )
                            dstv = osb[ct][:, h - g0].rearrange(
                                "p a (k w) -> p a k w", k=NBLK
                            )
                            for blk in range(NBLK):
                                src = ps[blk][:].rearrange("p (a w) -> p a w", a=2)
                                dst = dstv[:, :, blk, :]
                                if blk == 0:
                                    nc.vector.tensor_copy(dst, src)
                                else:
                                    nc.scalar.copy(dst, src)
                    for ct in range(2):
                        nc.scalar.dma_start(
                            out[ct * 128 : (ct + 1) * 128, S * g0 : S * g1, :],
                            osb[ct][:, : g1 - g0].rearrange("p g a c -> p (g a c)"),
                        )

            if iters == 1:
                body()
            else:
                with tc.For_i(0, iters, 1) as _i:
                    body(_i)
    nc.finalize()
    return nc


host_prep_v5 = host_prep_v2


# ---------------- v7: single-queue FIFO ordering (outs behind the band) ----------------


def build_program_v7(iters: int = 1, dt=F16, psbufs: int = 4, obufs: int = 3,
                     out_group: int = 5, bsplit=(2, 3, 4, 4, 6, 6),
                     fsplit=(9, 10, 10)):
    """v5 compute; all band chunks AND output DMAs ride the sync HWDGE queue
    (FIFO => every out-DMA waits for the full band, so the PE never starves),
    while the small feature stream lands in parallel on the scalar queue."""
    nc = bacc.Bacc(None, target_bir_lowering=False, debug=False)
    f_in = nc.dram_tensor("f", [128, NR, C], dt, kind="ExternalInput")
    b_in = nc.dram_tensor("b", [128, HC, KS, NCOL], dt, kind="ExternalInput")
    out = nc.dram_tensor("out", [C, S * HC, S * W], F16, kind="ExternalOutput")

    with tile.TileContext(nc) as tc:
        with (
            tc.tile_pool(name="fsb", bufs=1) as fpool,
            tc.tile_pool(name="bsb", bufs=1) as bpool,
            tc.tile_pool(name="osb", bufs=obufs) as opool,
            tc.tile_pool(name="ps0", bufs=psbufs, space="PSUM") as pspool0,
            tc.tile_pool(name="ps1", bufs=psbufs, space="PSUM") as pspool1,
        ):
            pspools = [pspool0, pspool1]

            def body(_=None):
                F_sb = fpool.tile([128, NR, C], dt)
                B_sb = bpool.tile([128, HC, KS, NCOL], dt)
                h0 = 0
                for bs in bsplit:
                    nc.sync.dma_start(B_sb[:, h0 : h0 + bs], b_in[:, h0 : h0 + bs])
                    h0 += bs
                r0 = 0
                for fs in fsplit:
                    nc.scalar.dma_start(F_sb[:, r0 : r0 + fs], f_in[:, r0 : r0 + fs])
                    r0 += fs
                G = out_group
                for g0 in range(0, HC, G):
                    g1 = min(g0 + G, HC)
                    osb = [
                        opool.tile([128, G, 2, NBLK * S * L], F16, name=f"o{ct}_{g0}")
                        for ct in range(2)
                    ]
                    for h in range(g0, g1):
                        for ct in range(2):
                            ps = [
                                pspools[blk].tile(
                                    [128, NCOL], F32, name=f"ps{blk}_{ct}_{h}",
                                    tag=f"psb{blk}",
                                )
                                for blk in range(NBLK)
                            ]
                            for dy in range(KS):
                                for blk in range(NBLK):
                                    lo = KB2 * blk
                                    nc.tensor.matmul(
                                        ps[blk][:, :],
                                        F_sb[lo : lo + KB2, h + dy,
                                             ct * 128 : (ct + 1) * 128],
                                        B_sb[lo : lo + KB2, h, dy, :],
                                        start=(dy == 0),
                                        stop=(dy == KS - 1),
                                    )
                            dstv = osb[ct][:, h - g0].rearrange(
                                "p a (k w) -> p a k w", k=NBLK
                            )
                            for blk in range(NBLK):
                                src = ps[blk][:].rearrange("p (a w) -> p a w", a=2)
                                dst = dstv[:, :, blk, :]
                                if blk == 0:
                                    nc.vector.tensor_copy(dst, src)
                                else:
                                    nc.scalar.copy(dst, src)
                    for ct in range(2):
                        nc.sync.dma_start(
                            out[ct * 128 : (ct + 1) * 128, S * g0 : S * g1, :],
                            osb[ct][:, : g1 - g0].rearrange("p g a c -> p (g a c)"),
                        )

            if iters == 1:
                body()
            else:
                with tc.For_i(0, iters, 1) as _i:
                    body(_i)
    nc.finalize()
    return nc


host_prep_v7 = host_prep_v2


# ---------------- v10: hybrid band build (DMA head/tail + GPSIMD scatter middle) ----------------
SC_LO, SC_HI = 2, 14   # rows built by local_scatter: h in [SC_LO, SC_HI)


def build_program_v10(iters: int = 1, dt=F16, psbufs: int = 4, obufs: int = 3,
                      groups=(5, 5, 5, 5, 5), scpair: int = 2):
    """Hybrid band sourcing: rows h0-1 and h14-24 arrive as host-built band
    via DMA (first matmul gated only by the small h0-1 chunk); rows h2-13 are
    built on-device by GPSIMD local_scatter from compact masks (d, 0.3 MB vs
    3.1 MB of band). A dependency-free dummy scatter at program start hoists
    the ~5us GPSIMD library load off the critical path. Output DMAs ride the
    sync queue FIFO behind the (now small) band stream."""
    nsc = SC_HI - SC_LO
    nc = bacc.Bacc(None, target_bir_lowering=False, debug=False)
    f_in = nc.dram_tensor("f", [128, NR, C], dt, kind="ExternalInput")
    b0_in = nc.dram_tensor("b0", [128, SC_LO, KS, NCOL], dt, kind="ExternalInput")
    bt_in = nc.dram_tensor("bt", [128, HC - SC_HI, KS, NCOL], dt,
                           kind="ExternalInput")
    d_in = nc.dram_tensor("d", [128, nsc, NIDX], dt, kind="ExternalInput")
    ix_in = nc.dram_tensor("idx", [128, scpair * NIDX], mybir.dt.int16,
                           kind="ExternalInput")
    out = nc.dram_tensor("out", [C, S * HC, S * W], F16, kind="ExternalOutput")

    with tile.TileContext(nc) as tc:
        with (
            tc.tile_pool(name="fsb", bufs=1) as fpool,
            tc.tile_pool(name="bsb", bufs=1) as bpool,
            tc.tile_pool(name="dsb", bufs=1) as dpool,
            tc.tile_pool(name="osb", bufs=obufs) as opool,
            tc.tile_pool(name="ps0", bufs=psbufs, space="PSUM") as pspool0,
            tc.tile_pool(name="ps1", bufs=psbufs, space="PSUM") as pspool1,
        ):
            pspools = [pspool0, pspool1]

            def body(_=None):
                F_sb = fpool.tile([128, NR, C], dt)
                B_sb = bpool.tile([128, HC, KS, NCOL], dt)
                D_sb = dpool.tile([128, nsc, NIDX], dt)
                I_sb = dpool.tile([128, scpair * NIDX], mybir.dt.int16, name="isb")
                dum = dpool.tile([128, 8], mybir.dt.int16, name="dum")
                # dependency-free dummy scatter: hoists the GPSIMD lib load
                nc.gpsimd.memset(dum[:], -1)
                nc.gpsimd.local_scatter(
                    dum[:, 0:2], dum[:, 2:6], dum[:, 2:6],
                    channels=128, num_elems=2, num_idxs=4,
                )
                # sync queue: first-rows band, idx, compact masks, tail band, outs
                nc.sync.dma_start(B_sb[:, :SC_LO], b0_in[:])
                nc.sync.dma_start(I_sb[:], ix_in[:])
                step = (nsc + scpair * 2 - 1) // (scpair * 2)
                for k in range(0, nsc, 4):
                    k1 = min(k + 4, nsc)
                    nc.sync.dma_start(D_sb[:, k:k1], d_in[:, k:k1])
                nt = HC - SC_HI
                for k0, k1 in ((0, nt // 2), (nt // 2, nt)):
                    nc.sync.dma_start(
                        B_sb[:, SC_HI + k0 : SC_HI + k1], bt_in[:, k0:k1]
                    )
                r0 = 0
                for fs in (9, 10, 10):
                    nc.scalar.dma_start(F_sb[:, r0 : r0 + fs], f_in[:, r0 : r0 + fs])
                    r0 += fs
                # scatter-built middle rows, scpair rows per op
                for h in range(SC_LO, SC_HI, scpair):
                    nc.gpsimd.local_scatter(
                        B_sb[:, h : h + scpair].rearrange("p g d c -> p (g d c)"),
                        D_sb[:, h - SC_LO : h - SC_LO + scpair].rearrange(
                            "p g c -> p (g c)"),
                        I_sb[:],
                        channels=128,
                        num_elems=scpair * KS * NCOL,
                        num_idxs=scpair * NIDX,
                    )
                g0 = 0
                maxg = max(groups)
                for G in groups:
                    g1 = g0 + G
                    osb = [
                        opool.tile([128, maxg, 2, NBLK * S * L], F16,
                                   name=f"o{ct}_{g0}")
                        for ct in range(2)
                    ]
                    for h in range(g0, g1):
                        for ct in range(2):
                            ps = [
                                pspools[blk].tile(
                                    [128, NCOL], F32, name=f"ps{blk}_{ct}_{h}",
                                    tag=f"psb{blk}",
                                )
                                for blk in range(NBLK)
                            ]
                            for dy in range(KS):
                                for blk in range(NBLK):
                                    lo = KB2 * blk
                                    nc.tensor.matmul(
                                        ps[blk][:, :],
                                        F_sb[lo : lo + KB2, h + dy,
                                             ct * 128 : (ct + 1) * 128],
                                        B_sb[lo : lo + KB2, h, dy, :],
                                        start=(dy == 0),
                                        stop=(dy == KS - 1),
                                    )
                            dstv = osb[ct][:, h - g0].rearrange(
                                "p a (k w) -> p a k w", k=NBLK
                            )
                            for blk in range(NBLK):
                                src = ps[blk][:].rearrange("p (a w) -> p a w", a=2)
                                dst = dstv[:, :, blk, :]
                                if blk == 0:
                                    nc.vector.tensor_copy(dst, src)
                                else:
                                    nc.scalar.copy(dst, src)
                    for ct in range(2):
                        nc.sync.dma_start(
                            out[ct * 128 : (ct + 1) * 128, S * g0 : S * g1, :],
                            osb[ct][:, : g1 - g0].rearrange("p g a c -> p (g a c)"),
                        )
                    g0 = g1

            if iters == 1:
                body()
            else:
                with tc.For_i(0, iters, 1) as _i:
                    body(_i)
    nc.finalize()
    return nc


def host_prep_v10(features: np.ndarray, masks: np.ndarray):
    """v10 inputs: f as v2; b0/bt = band rows [0,SC_LO) and [SC_HI,HC);
    d = pre-shifted compact masks for rows [SC_LO,SC_HI); idx = scatter table
    for `scpair` consecutive rows (second row offset by one band slab)."""
    f_hosts, b_hosts = host_prep_v2(features, masks)
    nsc = SC_HI - SC_LO
    wl_idx = np.arange(L)
    aux = []
    for core in range(8):
        n, q = divmod(core, 4)
        h0 = HC * q
        m7 = masks[n].reshape(KS, KS, H, S, W, S)[:, :, h0 : h0 + HC]
        d_host = np.zeros((128, nsc, KS, KS, 2, 2), np.float16)
        for blk in range(NBLK):
            for u in range(KS):
                src = m7[:, u, SC_LO:SC_HI, :, L * blk + wl_idx, :]  # [wl,dy,h,a,b]
                d_host[KB2 * blk + u + wl_idx, :, :, u, :, :] = (
                    src.transpose(0, 2, 1, 3, 4)
                )
        b_full = b_hosts[core].reshape(128, HC, KS, NCOL)
        aux.append({
            "b0": np.ascontiguousarray(b_full[:, :SC_LO]),
            "bt": np.ascontiguousarray(b_full[:, SC_HI:]),
            "d": d_host.reshape(128, nsc, NIDX),
        })
    idx = np.full((128, KS, KS, 2, 2), -1, np.int16)
    for blk in range(NBLK):
        for pp in range(KB):
            for u in range(KS):
                wl = pp - u
                if 0 <= wl < L:
                    for dy in range(KS):
                        for a in range(2):
                            for b in range(2):
                                idx[KB2 * blk + pp, dy, u, a, b] = (
                                    ((dy * 2 + a) * L + wl) * 2 + b
                                )
    idx1 = idx.reshape(128, NIDX)
    idx2 = np.concatenate([idx1, np.where(idx1 >= 0, idx1 + KS * NCOL, -1)],
                         axis=1).astype(np.int16)
    for core in range(8):
        aux[core]["idx"] = idx2
    return f_hosts, aux


# ---------------- v9: v7 retuned (no warmup, fewer chunks, tail options) ----------------


def build_program_v9(iters: int = 1, dt=F16, psbufs: int = 4, obufs: int = 3,
                     groups=(5, 5, 5, 5, 5), bsplit=(2, 3, 4, 8, 8),
                     fsplit=(9, 10, 10), warm: int = 0):
    """v7 with chunk counts tuned for the 11-semaphore DMA pool: 5 band
    chunks + 3 feature chunks issue up-front without semaphore recycling
    stalls; outputs FIFO behind the band on the sync queue."""
    nc = bacc.Bacc(None, target_bir_lowering=False, debug=False)
    f_in = nc.dram_tensor("f", [128, NR, C], dt, kind="ExternalInput")
    b_in = nc.dram_tensor("b", [128, HC, KS, NCOL], dt, kind="ExternalInput")
    out = nc.dram_tensor("out", [C, S * HC, S * W], F16, kind="ExternalOutput")

    with tile.TileContext(nc) as tc:
        with (
            tc.tile_pool(name="fsb", bufs=1) as fpool,
            tc.tile_pool(name="bsb", bufs=1) as bpool,
            tc.tile_pool(name="osb", bufs=obufs) as opool,
            tc.tile_pool(name="ps0", bufs=psbufs, space="PSUM") as pspool0,
            tc.tile_pool(name="ps1", bufs=psbufs, space="PSUM") as pspool1,
        ):
            pspools = [pspool0, pspool1]

            def body(_=None):
                F_sb = fpool.tile([128, NR, C], dt)
                B_sb = bpool.tile([128, HC, KS, NCOL], dt)
                if warm == -1:
                    # interleave: scalar carries F plus every other B chunk
                    r0 = fsplit[0]
                    nc.scalar.dma_start(F_sb[:, :r0], f_in[:, :r0])
                    h0 = 0
                    fi = 1
                    for bi, bs in enumerate(bsplit):
                        eng = nc.scalar if bi % 2 == 1 else nc.sync
                        eng.dma_start(B_sb[:, h0 : h0 + bs], b_in[:, h0 : h0 + bs])
                        h0 += bs
                        if bi % 2 == 1 and fi < len(fsplit):
                            fs = fsplit[fi]
                            nc.scalar.dma_start(
                                F_sb[:, r0 : r0 + fs], f_in[:, r0 : r0 + fs]
                            )
                            r0 += fs
                            fi += 1
                else:
                    h0 = 0
                    for bs in bsplit:
                        nc.sync.dma_start(
                            B_sb[:, h0 : h0 + bs], b_in[:, h0 : h0 + bs]
                        )
                        h0 += bs
                    r0 = 0
                    for fs in fsplit:
                        nc.scalar.dma_start(
                            F_sb[:, r0 : r0 + fs], f_in[:, r0 : r0 + fs]
                        )
                        r0 += fs
                if warm > 0:
                    # keep the PE busy through the DMA lead-in so the HAM
                    # clock gate is at 2.4 GHz when real matmuls start
                    wsb = fpool.tile([128, 128], dt, name="wsb")
                    nc.vector.memset(wsb[:], 0.0)
                    wps = pspool1.tile([128, 128], F32, name="warmps", tag="psb1")
                    for k in range(warm):
                        nc.tensor.matmul(
                            wps[:, :], wsb[:, :], wsb[:, :],
                            start=(k == 0), stop=(k == warm - 1),
                        )
                g0 = 0
                maxg = max(groups)
                for G in groups:
                    g1 = g0 + G
                    osb = [
                        opool.tile([128, maxg, 2, NBLK * S * L], F16,
                                   name=f"o{ct}_{g0}")
                        for ct in range(2)
                    ]
                    for h in range(g0, g1):
                        for ct in range(2):
                            ps = [
                                pspools[blk].tile(
                                    [128, NCOL], F32, name=f"ps{blk}_{ct}_{h}",
                                    tag=f"psb{blk}",
                                )
                                for blk in range(NBLK)
                            ]
                            for dy in range(KS):
                                for blk in range(NBLK):
                                    lo = KB2 * blk
                                    nc.tensor.matmul(
                                        ps[blk][:, :],
                                        F_sb[lo : lo + KB2, h + dy,
                                             ct * 128 : (ct + 1) * 128],
                                        B_sb[lo : lo + KB2, h, dy, :],
                                        start=(dy == 0),
                                        stop=(dy == KS - 1),
                                    )
                            dstv = osb[ct][:, h - g0].rearrange(
                                "p a (k w) -> p a k w", k=NBLK
                            )
                            for blk in range(NBLK):
                                src = ps[blk][:].rearrange("p (a w) -> p a w", a=2)
                                dst = dstv[:, :, blk, :]
                                if blk == 0:
                                    nc.vector.tensor_copy(dst, src)
                                else:
                                    nc.scalar.copy(dst, src)
                    for ct in range(2):
                        nc.sync.dma_start(
                            out[ct * 128 : (ct + 1) * 128, S * g0 : S * g1, :],
                            osb[ct][:, : g1 - g0].rearrange("p g a c -> p (g a c)"),
                        )
                    g0 = g1

            if iters == 1:
                body()
            else:
                with tc.For_i(0, iters, 1) as _i:
                    body(_i)
    nc.finalize()
    return nc


host_prep_v9 = host_prep_v2


# ---------------- v8: v7 + tiny lead chunks + PE warmup + finer out tail ----------------


def build_program_v8(iters: int = 1, dt=F16, psbufs: int = 4, obufs: int = 3,
                     groups=(5, 5, 5, 4, 3, 3), bsplit=(1, 2, 3, 4, 5, 5, 5),
                     fsplit=(5, 8, 8, 8), warm: int = 16):
    """v7 + (a) 1-row first band chunk so the first matmul fires ASAP,
    (b) dummy matmuls on scratch during the DMA lead-in to lift the PE HAM
    clock gate to 2.4 GHz before real work, (c) smaller trailing output
    groups so the final out-DMA after the last matmul is short."""
    nc = bacc.Bacc(None, target_bir_lowering=False, debug=False)
    f_in = nc.dram_tensor("f", [128, NR, C], dt, kind="ExternalInput")
    b_in = nc.dram_tensor("b", [128, HC, KS, NCOL], dt, kind="ExternalInput")
    out = nc.dram_tensor("out", [C, S * HC, S * W], F16, kind="ExternalOutput")

    with tile.TileContext(nc) as tc:
        with (
            tc.tile_pool(name="fsb", bufs=1) as fpool,
            tc.tile_pool(name="bsb", bufs=1) as bpool,
            tc.tile_pool(name="osb", bufs=obufs) as opool,
            tc.tile_pool(name="ps0", bufs=psbufs, space="PSUM") as pspool0,
            tc.tile_pool(name="ps1", bufs=psbufs, space="PSUM") as pspool1,
        ):
            pspools = [pspool0, pspool1]

            def body(_=None):
                F_sb = fpool.tile([128, NR, C], dt)
                B_sb = bpool.tile([128, HC, KS, NCOL], dt)
                if warm:
                    wsb = fpool.tile([128, 512], dt, name="wsb")
                    nc.vector.memset(wsb[:], 0.0)
                    wps = pspool0.tile([128, 512], F32, name="warmps", tag="psb0")
                    for k in range(warm):
                        nc.tensor.matmul(
                            wps[:, :], wsb[:, 0:128], wsb[:, :],
                            start=(k == 0), stop=(k == warm - 1),
                        )
                h0 = 0
                for bs in bsplit:
                    nc.sync.dma_start(B_sb[:, h0 : h0 + bs], b_in[:, h0 : h0 + bs])
                    h0 += bs
                r0 = 0
                for fs in fsplit:
                    nc.scalar.dma_start(F_sb[:, r0 : r0 + fs], f_in[:, r0 : r0 + fs])
                    r0 += fs
                g0 = 0
                for G in groups:
                    g1 = g0 + G
                    osb = [
                        opool.tile([128, groups[0], 2, NBLK * S * L], F16,
                                   name=f"o{ct}_{g0}")
                        for ct in range(2)
                    ]
                    for h in range(g0, g1):
                        for ct in range(2):
                            ps = [
                                pspools[blk].tile(
                                    [128, NCOL], F32, name=f"ps{blk}_{ct}_{h}",
                                    tag=f"psb{blk}",
                                )
                                for blk in range(NBLK)
                            ]
                            for dy in range(KS):
                                for blk in range(NBLK):
                                    lo = KB2 * blk
                                    nc.tensor.matmul(
                                        ps[blk][:, :],
                                        F_sb[lo : lo + KB2, h + dy,
                                             ct * 128 : (ct + 1) * 128],
                                        B_sb[lo : lo + KB2, h, dy, :],
                                        start=(dy == 0),
                                        stop=(dy == KS - 1),
                                    )
                            dstv = osb[ct][:, h - g0].rearrange(
                                "p a (k w) -> p a k w", k=NBLK
                            )
                            for blk in range(NBLK):
                                src = ps[blk][:].rearrange("p (a w) -> p a w", a=2)
                                dst = dstv[:, :, blk, :]
                                if blk == 0:
                                    nc.vector.tensor_copy(dst, src)
                                else:
                                    nc.scalar.copy(dst, src)
                    for ct in range(2):
                        nc.sync.dma_start(
                            out[ct * 128 : (ct + 1) * 128, S * g0 : S * g1, :],
                            osb[ct][:, : g1 - g0].rearrange("p g a c -> p (g a c)"),
                        )
                    g0 = g1

            if iters == 1:
                body()
            else:
                with tc.For_i(0, iters, 1) as _i:
                    body(_i)
    nc.finalize()
    return nc


host_prep_v8 = host_prep_v2


# ---------------- v6: on-device band build via GPSIMD local_scatter ----------------
NIDX = KS * KS * 2 * 2  # 100 scatter slots per partition: (dy, u=dx, a, b)


def build_program_v6(iters: int = 1, dt=F16, psbufs: int = 4, obufs: int = 3,
                     out_group: int = 5, dsplit=(5, 5, 5, 5, 5), fsplit=(9, 10, 10)):
    """v5 compute, but the banded mask matrices are built ON DEVICE:
    HBM carries only the compact masks pre-shifted per dx (d: 0.64 MB vs the
    6.4 MB host-built band) plus a 25.6 KB constant index table. One GPSIMD
    local_scatter per low-res row h places the 100 per-partition mask values
    onto the band diagonals (and zeroes the rest of that row's band slab)."""
    nc = bacc.Bacc(None, target_bir_lowering=False, debug=False)
    f_in = nc.dram_tensor("f", [128, NR, C], dt, kind="ExternalInput")
    d_in = nc.dram_tensor("d", [128, HC, NIDX], dt, kind="ExternalInput")
    ix_in = nc.dram_tensor("idx", [128, NIDX], mybir.dt.int16, kind="ExternalInput")
    out = nc.dram_tensor("out", [C, S * HC, S * W], F16, kind="ExternalOutput")

    with tile.TileContext(nc) as tc:
        with (
            tc.tile_pool(name="fsb", bufs=1) as fpool,
            tc.tile_pool(name="bsb", bufs=1) as bpool,
            tc.tile_pool(name="dsb", bufs=1) as dpool,
            tc.tile_pool(name="osb", bufs=obufs) as opool,
            tc.tile_pool(name="ps0", bufs=psbufs, space="PSUM") as pspool0,
            tc.tile_pool(name="ps1", bufs=psbufs, space="PSUM") as pspool1,
        ):
            pspools = [pspool0, pspool1]

            def body(_=None):
                F_sb = fpool.tile([128, NR, C], dt)
                B_sb = bpool.tile([128, HC, KS, NCOL], dt)
                D_sb = dpool.tile([128, HC, NIDX], dt)
                I_sb = dpool.tile([128, NIDX], mybir.dt.int16, name="isb")
                nc.sync.dma_start(I_sb[:], ix_in[:])
                h0 = 0
                for ds in dsplit:
                    nc.sync.dma_start(D_sb[:, h0 : h0 + ds], d_in[:, h0 : h0 + ds])
                    h0 += ds
                r0 = 0
                for fs in fsplit:
                    nc.scalar.dma_start(F_sb[:, r0 : r0 + fs], f_in[:, r0 : r0 + fs])
                    r0 += fs
                G = out_group
                for g0 in range(0, HC, G):
                    g1 = min(g0 + G, HC)
                    osb = [
                        opool.tile([128, G, 2, NBLK * S * L], F16, name=f"o{ct}_{g0}")
                        for ct in range(2)
                    ]
                    for h in range(g0, g1):
                        nc.gpsimd.local_scatter(
                            B_sb[:, h].rearrange("p d c -> p (d c)"),
                            D_sb[:, h],
                            I_sb[:],
                            channels=128,
                            num_elems=KS * NCOL,
                            num_idxs=NIDX,
                        )
                        for ct in range(2):
                            ps = [
                                pspools[blk].tile(
                                    [128, NCOL], F32, name=f"ps{blk}_{ct}_{h}",
                                    tag=f"psb{blk}",
                                )
                                for blk in range(NBLK)
                            ]
                            for dy in range(KS):
                                for blk in range(NBLK):
                                    lo = KB2 * blk
                                    nc.tensor.matmul(
                                        ps[blk][:, :],
                                        F_sb[lo : lo + KB2, h + dy,
                                             ct * 128 : (ct + 1) * 128],
                                        B_sb[lo : lo + KB2, h, dy, :],
                                        start=(dy == 0),
                                        stop=(dy == KS - 1),
                                    )
                            dstv = osb[ct][:, h - g0].rearrange(
                                "p a (k w) -> p a k w", k=NBLK
                            )
                            for blk in range(NBLK):
                                src = ps[blk][:].rearrange("p (a w) -> p a w", a=2)
                                dst = dstv[:, :, blk, :]
                                if blk == 0:
                                    nc.vector.tensor_copy(dst, src)
                                else:
                                    nc.scalar.copy(dst, src)
                    for ct in range(2):
                        nc.scalar.dma_start(
                            out[ct * 128 : (ct + 1) * 128, S * g0 : S * g1, :],
                            osb[ct][:, : g1 - g0].rearrange("p g a c -> p (g a c)"),
                        )

            if iters == 1:
                body()
            else:
                with tc.For_i(0, iters, 1) as _i:
                    body(_i)
    nc.finalize()
    return nc


def host_prep_v6(features: np.ndarray, masks: np.ndarray):
    """v6 inputs: F as in v2; d = per-dx partition-shifted compact masks
    [128, HC, (dy, u, a, b)]; idx = constant int16 scatter table [128, 100]."""
    f_hosts, _ = host_prep_v2(features, masks)
    d_hosts = []
    wl_idx = np.arange(L)
    for core in range(8):
        n, q = divmod(core, 4)
        h0 = HC * q
        m7 = masks[n].reshape(KS, KS, H, S, W, S)[:, :, h0 : h0 + HC]
        d_host = np.zeros((128, HC, KS, KS, 2, 2), np.float16)
        for blk in range(NBLK):
            for u in range(KS):
                src = m7[:, u, :, :, L * blk + wl_idx, :]  # [wl, dy, h, a, b]
                d_host[KB2 * blk + u + wl_idx, :, :, u, :, :] = (
                    src.transpose(0, 2, 1, 3, 4)
                )
        d_hosts.append(d_host.reshape(128, HC, NIDX))
    # scatter index table: for band partition q = 64*blk + p', slot (dy,u,a,b)
    # lands at free index ((dy*2 + a)*L + (p'-u))*2 + b of the (dy,a,wl,b) row,
    # or -1 (skipped) when wl = p'-u falls outside [0, L).
    idx = np.full((128, KS, KS, 2, 2), -1, np.int16)
    for blk in range(NBLK):
        for pp in range(KB):
            for u in range(KS):
                wl = pp - u
                if 0 <= wl < L:
                    for dy in range(KS):
                        for a in range(2):
                            for b in range(2):
                                idx[KB2 * blk + pp, dy, u, a, b] = (
                                    ((dy * 2 + a) * L + wl) * 2 + b
                                )
    idx_host = idx.reshape(128, NIDX)
    return f_hosts, [
        {"d": d_hosts[c], "idx": idx_host} for c in range(8)
    ]


# ---------------- v3: dy-pairs stacked in K (two taps per matmul) ----------------
NP3 = (KS + 1) // 2  # 3 matmuls per (h, blk): dy pairs (0,1), (2,3), (4,-)


def build_program_v3(iters: int = 1, dt=F16, copy_eng="both", parts="full",
                     psbufs: int = 3, obufs: int = 2, out_group: int = 5,
                     bchunks: int = 5, unroll: bool = False):
    """v3: K=128 = (dy-pair half j in {0,1}) x (w'' in [0,64)). The upper 64
    partitions hold a one-row-shifted copy of the features, so one matmul
    contracts two vertical taps. 300 matmuls of N=200, all tile_position
    (0,0), one PSUM bank per output row."""
    nc = bacc.Bacc(None, target_bir_lowering=False, debug=False)
    f_in = nc.dram_tensor("f", [128, NBLK, NR, C], dt, kind="ExternalInput")
    b_in = nc.dram_tensor("b", [128, NBLK, HC, NP3, NCOL], dt, kind="ExternalInput")
    out = nc.dram_tensor("out", [C, S * HC, S * W], F32, kind="ExternalOutput")

    with tile.TileContext(nc) as tc:
        with (
            tc.tile_pool(name="fsb", bufs=1) as fpool,
            tc.tile_pool(name="bsb", bufs=1) as bpool,
            tc.tile_pool(name="osb", bufs=obufs) as opool,
            tc.tile_pool(name="ps", bufs=psbufs, space="PSUM") as pspool,
        ):
            def body(_=None):
                F_sb = fpool.tile([128, NBLK, NR, C], dt)
                B_sb = bpool.tile([128, NBLK, HC, NP3, NCOL], dt)
                if parts != "nodmain":
                    nc.sync.dma_start(F_sb[:, :, : NR // 2], f_in[:, :, : NR // 2])
                    nc.sync.dma_start(F_sb[:, :, NR // 2 :], f_in[:, :, NR // 2 :])
                    bstep = (HC + bchunks - 1) // bchunks
                    for h0 in range(0, HC, bstep):
                        h1 = min(h0 + bstep, HC)
                        nc.sync.dma_start(B_sb[:, :, h0:h1], b_in[:, :, h0:h1])
                if parts == "dmain":
                    return
                G = out_group
                for ct in range(2):
                    for g0 in range(0, HC, G):
                        g1 = min(g0 + G, HC)
                        osb = opool.tile([128, G, 2, NBLK * S * L], F32)
                        for h in range(g0, g1):
                            ps = pspool.tile(
                                [128, NBLK * NCOL], F32, name=f"ps_{ct}_{h}",
                                tag="ps",
                            )
                            for blk in range(NBLK):
                                for p in range(NP3):
                                    nc.tensor.matmul(
                                        ps[:, blk * NCOL : (blk + 1) * NCOL],
                                        F_sb[:, blk, h + 2 * p,
                                             ct * 128 : (ct + 1) * 128],
                                        B_sb[:, blk, h, p, :],
                                        start=(blk == 0 and p == 0),
                                        stop=(blk == NBLK - 1 and p == NP3 - 1),
                                    )
                            if parts == "nocopy":
                                continue
                            # psum free layout (blk, a, w2l) -> dest (a, blk, w2l)
                            src = ps[:].rearrange("p (k a w) -> p a k w", k=NBLK, a=2)
                            dst = osb[:, h - g0].rearrange(
                                "p a (k w) -> p a k w", k=NBLK
                            )
                            if copy_eng == "vector" or (
                                copy_eng == "both" and h % 2 == 0
                            ):
                                nc.vector.tensor_copy(dst, src)
                            else:
                                nc.scalar.copy(dst, src)
                        if parts == "nocopy":
                            continue
                        nc.sync.dma_start(
                            out[ct * 128 : (ct + 1) * 128, S * g0 : S * g1, :],
                            osb[:, : g1 - g0].rearrange("p g a c -> p (g a c)"),
                        )

            if iters == 1:
                body()
            elif unroll:
                for _k in range(iters):
                    body(_k)
            else:
                with tc.For_i(0, iters, 1) as _i:
                    body(_i)
    nc.finalize()
    return nc


def host_prep_v3(features: np.ndarray, masks: np.ndarray):
    """v3 layouts: partition = 64*j + w''; j=1 half holds features shifted one
    row down (dy-pair trick). Separate windows per width block."""
    f_hosts, b_hosts = [], []
    padded = np.pad(features, ((0, 0), (0, 0), (R, R), (R, R)))
    wl_idx = np.arange(L)
    for core in range(8):
        n, q = divmod(core, 4)
        h0 = HC * q
        F_core = padded[n, :, h0 : h0 + NR, :]  # [C, 29, 104]
        F_host = np.zeros((128, NBLK, NR, C), np.float16)
        for blk in range(NBLK):
            win = F_core[:, :, L * blk : L * blk + KB].transpose(2, 1, 0)  # [54,29,C]
            F_host[:KB, blk] = win                      # j=0: rows r
            F_host[64 : 64 + KB, blk, : NR - 1] = win[:, 1:]  # j=1: rows r+1
        m7 = masks[n].reshape(KS, KS, H, S, W, S)[:, :, h0 : h0 + HC]
        B_host = np.zeros((128, NBLK, HC, NP3, 2, L, 2), np.float16)
        for blk in range(NBLK):
            for dx in range(KS):
                for dy in range(KS):
                    p, j = divmod(dy, 2)
                    src = m7[dy, dx, :, :, L * blk : L * blk + L, :]  # [h,a,wl,b]
                    B_host[64 * j + dx + wl_idx, blk, :, p, :, wl_idx, :] = (
                        src.transpose(2, 0, 1, 3)
                    )
        f_hosts.append(F_host)
        b_hosts.append(B_host.reshape(128, NBLK, HC, NP3, NCOL))
    return f_hosts, b_hosts


_NC_CACHE = {}

# Best HW-verified configuration (single-shot device exec ~51 us/core mean,
# ~56 us max across the 8 cores, vs ~76/82 us for the v2 baseline measured
# the same way):
#   - v2's banded-matmul compute (K=64, two concurrent PE row-group chains,
#     N=200 per matmul, PSUM-accumulated over the 5 vertical taps),
#   - fp16 output staging (halves the out-DMA to 5.12 MB/core; host upcasts),
#   - h-outer/ct-inner loop so each band chunk feeds 2x the matmuls,
#   - band chunks consumption-ordered on the sync HWDGE queue with a tiny
#     first chunk (first matmul fires ~10.5 us in), features on the scalar
#     queue, and all output DMAs FIFO'd on the sync queue BEHIND the band so
#     they can never starve the PE of mask data,
#   - finer trailing output groups to shorten the post-compute DMA tail.
_BEST_KW = dict(groups=(5, 5, 5, 5, 3, 2), bsplit=(1, 2, 4, 4, 7, 7),
                fsplit=(5, 9, 15))


def _get_program(iters: int = 1):
    if iters not in _NC_CACHE:
        _NC_CACHE[iters] = build_program_v9(iters, **_BEST_KW)
    return _NC_CACHE[iters]


def kernel(features: np.ndarray, masks: np.ndarray) -> np.ndarray:
    features = np.ascontiguousarray(features, dtype=np.float32)
    masks = np.ascontiguousarray(masks, dtype=np.float32)
    f_hosts, b_hosts = host_prep_v2(features, masks)
    in_maps = [{"f": f_hosts[c], "b": b_hosts[c]} for c in range(8)]
    nc = _get_program(1)
    res = run_bass_kernel_spmd(nc, in_maps, list(range(8)))
    out = np.empty((N, C, S * H, S * W), np.float32)
    for core in range(8):
        n, q = divmod(core, 4)
        out[n, :, S * HC * q : S * HC * (q + 1), :] = (
            res.results[core]["out"].astype(np.float32)
        )
    return out



# revision 19
# speedup vs baseline: 1.3801x; 1.0380x over previous
"""CARAFE-naive 2x content-aware upsampling on 8 Trainium2 NeuronCores.

Problem: features [2, 256, 100, 100] f32, masks [2, 25, 200, 200] f32
-> out [2, 256, 200, 200] f32, where each output pixel is a 25-tap (5x5)
weighted sum of the source neighborhood, weights shared across channels.

Strategy (per core = one (image n, row-quarter q) pair):
  The 25-tap contraction is cast as TensorE matmuls via a banded-matrix
  trick along the width axis. For one low-res output row h and width
  block of L=50 low-res columns, the contraction over the 5 horizontal
  taps is a matmul with contraction dim K = L+4 = 54 (the padded width
  window): out[c, (a, w2)] = sum_w' F[w', c] * Band[w', (a, w2)], where
  Band packs mask values on 5 diagonals (built host-side in numpy).
  The 5 vertical taps (dy) accumulate in PSUM across 5 matmuls.

  lhsT = transposed feature row slices (stationary), rhs = banded mask
  blocks. Both fp16 (PE runs fp16 at full rate; ~2^-11 rel precision).
  The two width blocks sit at SBUF partition bases 0/64 and run as two
  concurrent PE row-group chains, each accumulating into its own PSUM
  bank (build_program_v9, the shipping configuration).

Single-shot latency decisions (see _BEST_KW): fp16 output staging
(host upcasts to f32), h-outer/ct-inner loop order, band chunks
consumption-ordered on the sync HWDGE queue with output DMAs FIFO'd
behind them, features on the scalar queue, finer trailing out groups.

Host-side numpy does layout/packing only (transpose, pad, diagonal
scatter of masks into band matrices); all FLOPs run on the device.
"""

import numpy as np

import concourse.mybir as mybir
import concourse.tile as tile
from concourse import bacc
from concourse.bass_utils import run_bass_kernel_spmd

# problem constants
N, C, H, W = 2, 256, 100, 100
KS = 5        # kernel size
S = 2         # upsample scale
R = (KS - 1) // 2

# sharding / blocking constants
HC = H // 4       # 25 low-res rows per core (8 cores = 2 images x 4 quarters)
NR = HC + 2 * R   # 29 padded feature rows per core
NBLK = 2          # width blocks
L = W // NBLK     # 50 low-res columns per block
KB = L + KS - 1   # 54 = matmul contraction size
PBASE = 64        # SBUF partition base stride between blocks
NCOL = 2 * S * L  # 200 matmul N per block: (a in 2, w2l in 100)
F16 = mybir.dt.float16
F32 = mybir.dt.float32


def build_program(iters: int = 1, dt=F16, blks=(0, 1), copy_eng="both", parts="full",
                  in_chunks: int = 1, in_engines=("sync",)):
    """Build the per-core bass program. `iters`>1 wraps the whole compute in
    a hardware loop (used only for benchmarking slope timing)."""
    nc = bacc.Bacc(None, target_bir_lowering=False, debug=False)
    f_in = nc.dram_tensor("f", [KB, NBLK, NR, C], dt, kind="ExternalInput")
    b_in = nc.dram_tensor("b", [KB, NBLK, HC, KS, NCOL], dt, kind="ExternalInput")
    out = nc.dram_tensor("out", [C, S * HC, S * W], F32, kind="ExternalOutput")

    with tile.TileContext(nc) as tc:
        with (
            tc.tile_pool(name="fsb", bufs=1) as fpool,
            tc.tile_pool(name="bsb", bufs=1) as bpool,
            tc.tile_pool(name="osb", bufs=4) as opool,
            tc.tile_pool(name="ps", bufs=6, space="PSUM") as pspool,
        ):
            def body(_=None):
                F_sb = fpool.tile([KB, NBLK, NR, C], dt)
                B_sb = bpool.tile([KB, NBLK, HC, KS, NCOL], dt)
                if parts == "dmain128":
                    # DMA-bandwidth probe: same bytes, 108-partition layout
                    F2 = fpool.tile([KB * NBLK, NR, C], dt, name="F2")
                    B2 = bpool.tile([KB * NBLK, HC, KS, NCOL], dt, name="B2")
                    f2 = f_in[:].rearrange("k n r c -> (k n) r c")
                    b2 = b_in[:].rearrange("k n h d c -> (k n) h d c")
                    engs = [getattr(nc, e) for e in in_engines]
                    step = (KB * NBLK + in_chunks - 1) // in_chunks
                    for i, p0 in enumerate(range(0, KB * NBLK, step)):
                        p1 = min(p0 + step, KB * NBLK)
                        engs[i % len(engs)].dma_start(F2[p0:p1], f2[p0:p1])
                        engs[i % len(engs)].dma_start(B2[p0:p1], b2[p0:p1])
                    return
                if parts != "nodmain":
                    engs = [getattr(nc, e) for e in in_engines]
                    ei = 0
                    # split each input DMA into in_chunks along a free dim to
                    # engage more DMA queues in parallel
                    fstep = (NR + in_chunks - 1) // in_chunks
                    for r0 in range(0, NR, fstep):
                        r1 = min(r0 + fstep, NR)
                        engs[ei % len(engs)].dma_start(
                            F_sb[:, :, r0:r1], f_in[:, :, r0:r1]
                        )
                        ei += 1
                    bstep = (HC + in_chunks - 1) // in_chunks
                    for h0 in range(0, HC, bstep):
                        h1 = min(h0 + bstep, HC)
                        engs[ei % len(engs)].dma_start(
                            B_sb[:, :, h0:h1], b_in[:, :, h0:h1]
                        )
                        ei += 1
                if parts == "dmain":
                    return
                for ct in range(2):
                    psums = {}
                    for r in range(NR):
                        for blk in blks:
                            lhsT = F_sb[:, blk, r, ct * 128 : (ct + 1) * 128]
                            for dy in range(KS):
                                h = r - dy
                                if not (0 <= h < HC):
                                    continue
                                if dy == 0 and blk == blks[0]:
                                    psums[h] = pspool.tile(
                                        [128, NBLK * NCOL],
                                        F32,
                                        name=f"ps{ct}_{h}",
                                        tag="ps",
                                    )
                                # One accumulation group per PSUM bank: start
                                # zeroes the whole 2KB zero-region, so only
                                # the first matmul of the tile starts and only
                                # the last one stops.
                                nc.tensor.matmul(
                                    psums[h][:, blk * NCOL : (blk + 1) * NCOL],
                                    lhsT,
                                    B_sb[:, blk, h, dy, :],
                                    start=(dy == 0 and blk == blks[0]),
                                    stop=(dy == KS - 1 and blk == blks[-1]),
                                )
                        h_done = r - (KS - 1)
                        if h_done >= 0 and parts in ("full", "nodmain"):
                            ps = psums.pop(h_done)
                            osb = opool.tile([128, 2, NBLK, S * L], F32)
                            # psum free layout (blk, a, w2l) -> (a, blk, w2l)
                            src = ps[:].rearrange(
                                "p (k a w) -> p a k w", k=NBLK, a=2
                            )
                            if copy_eng == "vector" or (copy_eng == "both" and h_done % 2 == 0):
                                nc.vector.tensor_copy(osb[:], src)
                            else:
                                nc.scalar.copy(osb[:], src)
                            nc.sync.dma_start(
                                out[ct * 128 : (ct + 1) * 128,
                                    S * h_done : S * h_done + 2, :],
                                osb[:].rearrange("p a k w -> p a (k w)"),
                            )

            if iters == 1:
                body()
            else:
                with tc.For_i(0, iters, 1) as _i:
                    body(_i)
    nc.finalize()
    return nc


def host_prep(features: np.ndarray, masks: np.ndarray):
    """Pack per-core fp16 inputs: transposed padded feature rows and banded
    mask matrices. Pure layout work (no arithmetic beyond dtype cast)."""
    f_hosts, b_hosts = [], []
    padded = np.pad(features, ((0, 0), (0, 0), (R, R), (R, R)))  # [N,C,H+4,W+4]
    wl_idx = np.arange(L)
    for core in range(8):
        n, q = divmod(core, 4)
        h0 = HC * q
        F_core = padded[n, :, h0 : h0 + NR, :]  # [C, 29, 104]
        F_host = np.zeros((KB, NBLK, NR, C), np.float16)
        for blk in range(NBLK):
            F_host[:, blk] = F_core[:, :, L * blk : L * blk + KB].transpose(2, 1, 0)
        # masks[n]: [25, 200, 200] -> [dy, dx, h, a, w, b]
        m7 = masks[n].reshape(KS, KS, H, S, W, S)[:, :, h0 : h0 + HC]
        B_host = np.zeros((KB, NBLK, HC, KS, 2, L, 2), np.float16)
        for blk in range(NBLK):
            for dx in range(KS):
                src = m7[:, dx, :, :, L * blk : L * blk + L, :]  # [dy,h,a,wl,b]
                B_host[dx + wl_idx, blk, :, :, :, wl_idx, :] = (
                    src.transpose(3, 1, 0, 2, 4)
                )
        f_hosts.append(F_host)
        b_hosts.append(B_host.reshape(KB, NBLK, HC, KS, NCOL))
    return f_hosts, b_hosts


# ---------------- v2: 128-partition layout, per-block PSUM banks ----------------
KB2 = 64  # padded contraction size (54 useful + 10 zero rows) -> blocks at 0/64


def build_program_v2(iters: int = 1, dt=F16, copy_eng="both", parts="full",
                     psbufs: int = 3, obufs: int = 2, out_group: int = 5,
                     bchunks: int = 5):
    """v2: both width blocks packed on 128 partitions (bases 0/64), each block
    accumulating into its own PSUM bank (documented-safe row-tiling pattern).
    dy-inner loop: weights reload per matmul but the two block chains run
    concurrently on different PE row groups."""
    nc = bacc.Bacc(None, target_bir_lowering=False, debug=False)
    f_in = nc.dram_tensor("f", [128, NR, C], dt, kind="ExternalInput")
    b_in = nc.dram_tensor("b", [128, HC, KS, NCOL], dt, kind="ExternalInput")
    out = nc.dram_tensor("out", [C, S * HC, S * W], F32, kind="ExternalOutput")

    with tile.TileContext(nc) as tc:
        with (
            tc.tile_pool(name="fsb", bufs=1) as fpool,
            tc.tile_pool(name="bsb", bufs=1) as bpool,
            tc.tile_pool(name="osb", bufs=obufs) as opool,
            tc.tile_pool(name="ps0", bufs=psbufs, space="PSUM") as pspool0,
            tc.tile_pool(name="ps1", bufs=psbufs, space="PSUM") as pspool1,
        ):
            pspools = [pspool0, pspool1]

            def body(_=None):
                F_sb = fpool.tile([128, NR, C], dt)
                B_sb = bpool.tile([128, HC, KS, NCOL], dt)
                if parts != "nodmain":
                    # chunked input DMAs: lets matmuls start after chunk 0
                    nc.sync.dma_start(F_sb[:, : NR // 2], f_in[:, : NR // 2])
                    nc.sync.dma_start(F_sb[:, NR // 2 :], f_in[:, NR // 2 :])
                    bstep = (HC + bchunks - 1) // bchunks
                    for h0 in range(0, HC, bstep):
                        h1 = min(h0 + bstep, HC)
                        nc.sync.dma_start(B_sb[:, h0:h1], b_in[:, h0:h1])
                if parts == "dmain":
                    return
                G = out_group
                for ct in range(2):
                    for g0 in range(0, HC, G):
                        g1 = min(g0 + G, HC)
                        osb = opool.tile([128, G, 2, NBLK * S * L], F32)
                        for h in range(g0, g1):
                            ps = [
                                pspools[blk].tile(
                                    [128, NCOL], F32, name=f"ps{blk}_{ct}_{h}",
                                    tag=f"psb{blk}",
                                )
                                for blk in range(NBLK)
                            ]
                            for dy in range(KS):
                                for blk in range(NBLK):
                                    lo = KB2 * blk
                                    nc.tensor.matmul(
                                        ps[blk][:, :],
                                        F_sb[lo : lo + KB2, h + dy,
                                             ct * 128 : (ct + 1) * 128],
                                        B_sb[lo : lo + KB2, h, dy, :],
                                        start=(dy == 0),
                                        stop=(dy == KS - 1),
                                    )
                            if parts == "nocopy":
                                continue
                            # osb free layout per h: (a, blk, w2l) built from the
                            # two psum tiles; dest dims [2, (blk, 100)]
                            dstv = osb[:, h - g0].rearrange(
                                "p a (k w) -> p a k w", k=NBLK
                            )
                            for blk in range(NBLK):
                                src = ps[blk][:].rearrange("p (a w) -> p a w", a=2)
                                dst = dstv[:, :, blk, :]
                                if copy_eng == "vector" or (
                                    copy_eng == "both" and blk == 0
                                ):
                                    nc.vector.tensor_copy(dst, src)
                                else:
                                    nc.scalar.copy(dst, src)
                        if parts == "nocopy":
                            continue
                        nc.sync.dma_start(
                            out[ct * 128 : (ct + 1) * 128,
                                S * g0 : S * g1, :],
                            osb[:, : g1 - g0].rearrange("p g a c -> p (g a c)"),
                        )

            if iters == 1:
                body()
            else:
                with tc.For_i(0, iters, 1) as _i:
                    body(_i)
    nc.finalize()
    return nc


def host_prep_v2(features: np.ndarray, masks: np.ndarray):
    """v2 layouts: [128, ...] with partition = 64*blk + w'' (w'' in [0,54))."""
    f_hosts, b_hosts = [], []
    padded = np.pad(features, ((0, 0), (0, 0), (R, R), (R, R)))
    wl_idx = np.arange(L)
    for core in range(8):
        n, q = divmod(core, 4)
        h0 = HC * q
        F_core = padded[n, :, h0 : h0 + NR, :]  # [C, 29, 104]
        F_host = np.zeros((128, NR, C), np.float16)
        for blk in range(NBLK):
            F_host[KB2 * blk : KB2 * blk + KB] = (
                F_core[:, :, L * blk : L * blk + KB].transpose(2, 1, 0)
            )
        m7 = masks[n].reshape(KS, KS, H, S, W, S)[:, :, h0 : h0 + HC]
        B_host = np.zeros((128, HC, KS, 2, L, 2), np.float16)
        for blk in range(NBLK):
            for dx in range(KS):
                src = m7[:, dx, :, :, L * blk : L * blk + L, :]  # [dy,h,a,wl,b]
                B_host[KB2 * blk + dx + wl_idx, :, :, :, wl_idx, :] = (
                    src.transpose(3, 1, 0, 2, 4)
                )
        f_hosts.append(F_host)
        b_hosts.append(B_host.reshape(128, HC, KS, NCOL))
    return f_hosts, b_hosts


# ---------------- v4: v2 + fp16 output (halves the out-DMA bytes) ----------------


def build_program_v4(iters: int = 1, dt=F16, parts="full",
                     psbufs: int = 3, obufs: int = 2, out_group: int = 5,
                     bchunks: int = 5):
    """v2 compute structure, but the PSUM->SBUF copy downcasts to fp16 and the
    output DRAM tensor is fp16 (host upcasts). Out-DMA drops 10.24 -> 5.12 MB."""
    nc = bacc.Bacc(None, target_bir_lowering=False, debug=False)
    f_in = nc.dram_tensor("f", [128, NR, C], dt, kind="ExternalInput")
    b_in = nc.dram_tensor("b", [128, HC, KS, NCOL], dt, kind="ExternalInput")
    out = nc.dram_tensor("out", [C, S * HC, S * W], F16, kind="ExternalOutput")

    with tile.TileContext(nc) as tc:
        with (
            tc.tile_pool(name="fsb", bufs=1) as fpool,
            tc.tile_pool(name="bsb", bufs=1) as bpool,
            tc.tile_pool(name="osb", bufs=obufs) as opool,
            tc.tile_pool(name="ps0", bufs=psbufs, space="PSUM") as pspool0,
            tc.tile_pool(name="ps1", bufs=psbufs, space="PSUM") as pspool1,
        ):
            pspools = [pspool0, pspool1]

            def body(_=None):
                F_sb = fpool.tile([128, NR, C], dt)
                B_sb = bpool.tile([128, HC, KS, NCOL], dt)
                if parts != "nodmain":
                    nc.sync.dma_start(F_sb[:, : NR // 2], f_in[:, : NR // 2])
                    nc.sync.dma_start(F_sb[:, NR // 2 :], f_in[:, NR // 2 :])
                    bstep = (HC + bchunks - 1) // bchunks
                    for h0 in range(0, HC, bstep):
                        h1 = min(h0 + bstep, HC)
                        nc.sync.dma_start(B_sb[:, h0:h1], b_in[:, h0:h1])
                if parts == "dmain":
                    return
                G = out_group
                for ct in range(2):
                    for g0 in range(0, HC, G):
                        g1 = min(g0 + G, HC)
                        osb = opool.tile([128, G, 2, NBLK * S * L], F16)
                        for h in range(g0, g1):
                            ps = [
                                pspools[blk].tile(
                                    [128, NCOL], F32, name=f"ps{blk}_{ct}_{h}",
                                    tag=f"psb{blk}",
                                )
                                for blk in range(NBLK)
                            ]
                            for dy in range(KS):
                                for blk in range(NBLK):
                                    lo = KB2 * blk
                                    nc.tensor.matmul(
                                        ps[blk][:, :],
                                        F_sb[lo : lo + KB2, h + dy,
                                             ct * 128 : (ct + 1) * 128],
                                        B_sb[lo : lo + KB2, h, dy, :],
                                        start=(dy == 0),
                                        stop=(dy == KS - 1),
                                    )
                            dstv = osb[:, h - g0].rearrange(
                                "p a (k w) -> p a k w", k=NBLK
                            )
                            for blk in range(NBLK):
                                src = ps[blk][:].rearrange("p (a w) -> p a w", a=2)
                                dst = dstv[:, :, blk, :]
                                if blk == 0:
                                    nc.vector.tensor_copy(dst, src)
                                else:
                                    nc.scalar.copy(dst, src)
                        nc.sync.dma_start(
                            out[ct * 128 : (ct + 1) * 128,
                                S * g0 : S * g1, :],
                            osb[:, : g1 - g0].rearrange("p g a c -> p (g a c)"),
                        )

            if iters == 1:
                body()
            else:
                with tc.For_i(0, iters, 1) as _i:
                    body(_i)
    nc.finalize()
    return nc


host_prep_v4 = host_prep_v2


# ---------------- v5: v4 + DMA orchestration for single-shot latency ----------------


def build_program_v5(iters: int = 1, dt=F16, psbufs: int = 4, obufs: int = 3,
                     out_group: int = 5,
                     bsplit=(3, 3, 4, 4, 5, 6), fsplit=(9, 10, 10)):
    """v2 compute + fp16 out + latency-ordered DMAs.

    - h-outer / ct-inner loop: each mask chunk feeds 2x the matmuls, so the
      mask stream only needs half the bandwidth to stay ahead of the PE.
    - b chunks land on the sync HWDGE queue in consumption order (small
      first chunk -> first matmul starts early); f chunks and all output
      DMAs ride the scalar HWDGE queue so outputs never queue behind masks.
    """
    nc = bacc.Bacc(None, target_bir_lowering=False, debug=False)
    f_in = nc.dram_tensor("f", [128, NR, C], dt, kind="ExternalInput")
    b_in = nc.dram_tensor("b", [128, HC, KS, NCOL], dt, kind="ExternalInput")
    out = nc.dram_tensor("out", [C, S * HC, S * W], F16, kind="ExternalOutput")

    with tile.TileContext(nc) as tc:
        with (
            tc.tile_pool(name="fsb", bufs=1) as fpool,
            tc.tile_pool(name="bsb", bufs=1) as bpool,
            tc.tile_pool(name="osb", bufs=obufs) as opool,
            tc.tile_pool(name="ps0", bufs=psbufs, space="PSUM") as pspool0,
            tc.tile_pool(name="ps1", bufs=psbufs, space="PSUM") as pspool1,
        ):
            pspools = [pspool0, pspool1]

            def body(_=None):
                F_sb = fpool.tile([128, NR, C], dt)
                B_sb = bpool.tile([128, HC, KS, NCOL], dt)
                # interleave issue order: B0, F0 first (gates the first MM)
                h0 = 0
                nc.sync.dma_start(B_sb[:, : bsplit[0]], b_in[:, : bsplit[0]])
                r0 = 0
                nc.scalar.dma_start(
                    F_sb[:, : fsplit[0]], f_in[:, : fsplit[0]]
                )
                h0 += bsplit[0]
                for bs in bsplit[1:]:
                    nc.sync.dma_start(B_sb[:, h0 : h0 + bs], b_in[:, h0 : h0 + bs])
                    h0 += bs
                r0 += fsplit[0]
                for fs in fsplit[1:]:
                    nc.scalar.dma_start(
                        F_sb[:, r0 : r0 + fs], f_in[:, r0 : r0 + fs]
                    )
                    r0 += fs
                G = out_group
                for g0 in range(0, HC, G):
                    g1 = min(g0 + G, HC)
                    osb = [
                        opool.tile([128, G, 2, NBLK * S * L], F16, name=f"o{ct}_{g0}")
                        for ct in range(2)
                    ]
                    for h in range(g0, g1):
                        for ct in range(2):
                            ps = [
                                pspools[blk].tile(
                                    [128, NCOL], F32, name=f"ps{blk}_{ct}_{h}",
                                    tag=f"psb{blk}",
                                )
                                for blk in range(NBLK)
                            ]
                            for dy in range(KS):
                                for blk in range(NBLK):
                                    lo = KB2 * blk
                                    nc.tensor.matmul(
                                        ps[blk][:, :],
                                        F_sb[lo : lo + KB2, h + dy,
                                             ct * 128 : (ct + 1) * 128],
                                        B_sb[lo : lo + KB2, h, dy, :],
                                        start=(dy == 0),
                                        stop=(dy == KS - 1),
                                    )
                            dstv = osb[ct][:, h - g0].rearrange(
                                "p a (k w) -> p a k w", k=NBLK
                            )
                            for blk in range(NBLK):
                                src = ps[blk][:].rearrange("p (a w) -> p a w", a=2)
                                dst = dstv[:, :, blk, :]
                                if blk == 0:
                                    nc.vector.tensor_copy(dst, src)
                                else:
                                    nc.scalar.copy(dst, src)
                    for ct in range(2):
                        nc.scalar.dma_start(
                            out[ct * 128 : (ct + 1) * 128, S * g0 : S * g1, :],
                            osb[ct][:, : g1 - g0].rearrange("p g a c -> p (g a c)"),
                        )

            if iters == 1:
                body()
            else:
                with tc.For_i(0, iters, 1) as _i:
                    body(_i)
    nc.finalize()
    return nc


host_prep_v5 = host_prep_v2


# ---------------- v7: single-queue FIFO ordering (outs behind the band) ----------------


def build_program_v7(iters: int = 1, dt=F16, psbufs: int = 4, obufs: int = 3,
                     out_group: int = 5, bsplit=(2, 3, 4, 4, 6, 6),
                     fsplit=(9, 10, 10)):
    """v5 compute; all band chunks AND output DMAs ride the sync HWDGE queue
    (FIFO => every out-DMA waits for the full band, so the PE never starves),
    while the small feature stream lands in parallel on the scalar queue."""
    nc = bacc.Bacc(None, target_bir_lowering=False, debug=False)
    f_in = nc.dram_tensor("f", [128, NR, C], dt, kind="ExternalInput")
    b_in = nc.dram_tensor("b", [128, HC, KS, NCOL], dt, kind="ExternalInput")
    out = nc.dram_tensor("out", [C, S * HC, S * W], F16, kind="ExternalOutput")

    with tile.TileContext(nc) as tc:
        with (
            tc.tile_pool(name="fsb", bufs=1) as fpool,
            tc.tile_pool(name="bsb", bufs=1) as bpool,
            tc.tile_pool(name="osb", bufs=obufs) as opool,
            tc.tile_pool(name="ps0", bufs=psbufs, space="PSUM") as pspool0,
            tc.tile_pool(name="ps1", bufs=psbufs, space="PSUM") as pspool1,
        ):
            pspools = [pspool0, pspool1]

            def body(_=None):
                F_sb = fpool.tile([128, NR, C], dt)
                B_sb = bpool.tile([128, HC, KS, NCOL], dt)
                h0 = 0
                for bs in bsplit:
                    nc.sync.dma_start(B_sb[:, h0 : h0 + bs], b_in[:, h0 : h0 + bs])
                    h0 += bs
                r0 = 0
                for fs in fsplit:
                    nc.scalar.dma_start(F_sb[:, r0 : r0 + fs], f_in[:, r0 : r0 + fs])
                    r0 += fs
                G = out_group
                for g0 in range(0, HC, G):
                    g1 = min(g0 + G, HC)
                    osb = [
                        opool.tile([128, G, 2, NBLK * S * L], F16, name=f"o{ct}_{g0}")
                        for ct in range(2)
                    ]
                    for h in range(g0, g1):
                        for ct in range(2):
                            ps = [
                                pspools[blk].tile(
                                    [128, NCOL], F32, name=f"ps{blk}_{ct}_{h}",
                                    tag=f"psb{blk}",
                                )
                                for blk in range(NBLK)
                            ]
                            for dy in range(KS):
                                for blk in range(NBLK):
                                    lo = KB2 * blk
                                    nc.tensor.matmul(
                                        ps[blk][:, :],
                                        F_sb[lo : lo + KB2, h + dy,
                                             ct * 128 : (ct + 1) * 128],
                                        B_sb[lo : lo + KB2, h, dy, :],
                                        start=(dy == 0),
                                        stop=(dy == KS - 1),
                                    )
                            dstv = osb[ct][:, h - g0].rearrange(
                                "p a (k w) -> p a k w", k=NBLK
                            )
                            for blk in range(NBLK):
                                src = ps[blk][:].rearrange("p (a w) -> p a w", a=2)
                                dst = dstv[:, :, blk, :]
                                if blk == 0:
                                    nc.vector.tensor_copy(dst, src)
                                else:
                                    nc.scalar.copy(dst, src)
                    for ct in range(2):
                        nc.sync.dma_start(
                            out[ct * 128 : (ct + 1) * 128, S * g0 : S * g1, :],
                            osb[ct][:, : g1 - g0].rearrange("p g a c -> p (g a c)"),
                        )

            if iters == 1:
                body()
            else:
                with tc.For_i(0, iters, 1) as _i:
                    body(_i)
    nc.finalize()
    return nc


host_prep_v7 = host_prep_v2


# ---------------- v10: hybrid band build (DMA head/tail + GPSIMD scatter middle) ----------------
SC_LO, SC_HI = 2, 14   # rows built by local_scatter: h in [SC_LO, SC_HI)


def build_program_v10(iters: int = 1, dt=F16, psbufs: int = 4, obufs: int = 3,
                      groups=(5, 5, 5, 5, 5), scpair: int = 2):
    """Hybrid band sourcing: rows h0-1 and h14-24 arrive as host-built band
    via DMA (first matmul gated only by the small h0-1 chunk); rows h2-13 are
    built on-device by GPSIMD local_scatter from compact masks (d, 0.3 MB vs
    3.1 MB of band). A dependency-free dummy scatter at program start hoists
    the ~5us GPSIMD library load off the critical path. Output DMAs ride the
    sync queue FIFO behind the (now small) band stream."""
    nsc = SC_HI - SC_LO
    nc = bacc.Bacc(None, target_bir_lowering=False, debug=False)
    f_in = nc.dram_tensor("f", [128, NR, C], dt, kind="ExternalInput")
    b0_in = nc.dram_tensor("b0", [128, SC_LO, KS, NCOL], dt, kind="ExternalInput")
    bt_in = nc.dram_tensor("bt", [128, HC - SC_HI, KS, NCOL], dt,
                           kind="ExternalInput")
    d_in = nc.dram_tensor("d", [128, nsc, NIDX], dt, kind="ExternalInput")
    ix_in = nc.dram_tensor("idx", [128, scpair * NIDX], mybir.dt.int16,
                           kind="ExternalInput")
    out = nc.dram_tensor("out", [C, S * HC, S * W], F16, kind="ExternalOutput")

    with tile.TileContext(nc) as tc:
        with (
            tc.tile_pool(name="fsb", bufs=1) as fpool,
            tc.tile_pool(name="bsb", bufs=1) as bpool,
            tc.tile_pool(name="dsb", bufs=1) as dpool,
            tc.tile_pool(name="osb", bufs=obufs) as opool,
            tc.tile_pool(name="ps0", bufs=psbufs, space="PSUM") as pspool0,
            tc.tile_pool(name="ps1", bufs=psbufs, space="PSUM") as pspool1,
        ):
            pspools = [pspool0, pspool1]

            def body(_=None):
                F_sb = fpool.tile([128, NR, C], dt)
                B_sb = bpool.tile([128, HC, KS, NCOL], dt)
                D_sb = dpool.tile([128, nsc, NIDX], dt)
                I_sb = dpool.tile([128, scpair * NIDX], mybir.dt.int16, name="isb")
                dum = dpool.tile([128, 8], mybir.dt.int16, name="dum")
                # dependency-free dummy scatter: hoists the GPSIMD lib load
                nc.gpsimd.memset(dum[:], -1)
                nc.gpsimd.local_scatter(
                    dum[:, 0:2], dum[:, 2:6], dum[:, 2:6],
                    channels=128, num_elems=2, num_idxs=4,
                )
                # sync queue: first-rows band, idx, compact masks, tail band, outs
                nc.sync.dma_start(B_sb[:, :SC_LO], b0_in[:])
                nc.sync.dma_start(I_sb[:], ix_in[:])
                step = (nsc + scpair * 2 - 1) // (scpair * 2)
                for k in range(0, nsc, 4):
                    k1 = min(k + 4, nsc)
                    nc.sync.dma_start(D_sb[:, k:k1], d_in[:, k:k1])
                nt = HC - SC_HI
                for k0, k1 in ((0, nt // 2), (nt // 2, nt)):
                    nc.sync.dma_start(
                        B_sb[:, SC_HI + k0 : SC_HI + k1], bt_in[:, k0:k1]
                    )
                r0 = 0
                for fs in (9, 10, 10):
                    nc.scalar.dma_start(F_sb[:, r0 : r0 + fs], f_in[:, r0 : r0 + fs])
                    r0 += fs
                # scatter-built middle rows, scpair rows per op
                for h in range(SC_LO, SC_HI, scpair):
                    nc.gpsimd.local_scatter(
                        B_sb[:, h : h + scpair].rearrange("p g d c -> p (g d c)"),
                        D_sb[:, h - SC_LO : h - SC_LO + scpair].rearrange(
                            "p g c -> p (g c)"),
                        I_sb[:],
                        channels=128,
                        num_elems=scpair * KS * NCOL,
                        num_idxs=scpair * NIDX,
                    )
                g0 = 0
                maxg = max(groups)
                for G in groups:
                    g1 = g0 + G
                    osb = [
                        opool.tile([128, maxg, 2, NBLK * S * L], F16,
                                   name=f"o{ct}_{g0}")
                        for ct in range(2)
                    ]
                    for h in range(g0, g1):
                        for ct in range(2):
                            ps = [
                                pspools[blk].tile(
                                    [128, NCOL], F32, name=f"ps{blk}_{ct}_{h}",
                                    tag=f"psb{blk}",
                                )
                                for blk in range(NBLK)
                            ]
                            for dy in range(KS):
                                for blk in range(NBLK):
                                    lo = KB2 * blk
                                    nc.tensor.matmul(
                                        ps[blk][:, :],
                                        F_sb[lo : lo + KB2, h + dy,
                                             ct * 128 : (ct + 1) * 128],
                                        B_sb[lo : lo + KB2, h, dy, :],
                                        start=(dy == 0),
                                        stop=(dy == KS - 1),
                                    )
                            dstv = osb[ct][:, h - g0].rearrange(
                                "p a (k w) -> p a k w", k=NBLK
                            )
                            for blk in range(NBLK):
                                src = ps[blk][:].rearrange("p (a w) -> p a w", a=2)
                                dst = dstv[:, :, blk, :]
                                if blk == 0:
                                    nc.vector.tensor_copy(dst, src)
                                else:
                                    nc.scalar.copy(dst, src)
                    for ct in range(2):
                        nc.sync.dma_start(
                            out[ct * 128 : (ct + 1) * 128, S * g0 : S * g1, :],
                            osb[ct][:, : g1 - g0].rearrange("p g a c -> p (g a c)"),
                        )
                    g0 = g1

            if iters == 1:
                body()
            else:
                with tc.For_i(0, iters, 1) as _i:
                    body(_i)
    nc.finalize()
    return nc


def host_prep_v10(features: np.ndarray, masks: np.ndarray):
    """v10 inputs: f as v2; b0/bt = band rows [0,SC_LO) and [SC_HI,HC);
    d = pre-shifted compact masks for rows [SC_LO,SC_HI); idx = scatter table
    for `scpair` consecutive rows (second row offset by one band slab)."""
    f_hosts, b_hosts = host_prep_v2(features, masks)
    nsc = SC_HI - SC_LO
    wl_idx = np.arange(L)
    aux = []
    for core in range(8):
        n, q = divmod(core, 4)
        h0 = HC * q
        m7 = masks[n].reshape(KS, KS, H, S, W, S)[:, :, h0 : h0 + HC]
        d_host = np.zeros((128, nsc, KS, KS, 2, 2), np.float16)
        for blk in range(NBLK):
            for u in range(KS):
                src = m7[:, u, SC_LO:SC_HI, :, L * blk + wl_idx, :]  # [wl,dy,h,a,b]
                d_host[KB2 * blk + u + wl_idx, :, :, u, :, :] = (
                    src.transpose(0, 2, 1, 3, 4)
                )
        b_full = b_hosts[core].reshape(128, HC, KS, NCOL)
        aux.append({
            "b0": np.ascontiguousarray(b_full[:, :SC_LO]),
            "bt": np.ascontiguousarray(b_full[:, SC_HI:]),
            "d": d_host.reshape(128, nsc, NIDX),
        })
    idx = np.full((128, KS, KS, 2, 2), -1, np.int16)
    for blk in range(NBLK):
        for pp in range(KB):
            for u in range(KS):
                wl = pp - u
                if 0 <= wl < L:
                    for dy in range(KS):
                        for a in range(2):
                            for b in range(2):
                                idx[KB2 * blk + pp, dy, u, a, b] = (
                                    ((dy * 2 + a) * L + wl) * 2 + b
                                )
    idx1 = idx.reshape(128, NIDX)
    idx2 = np.concatenate([idx1, np.where(idx1 >= 0, idx1 + KS * NCOL, -1)],
                         axis=1).astype(np.int16)
    for core in range(8):
        aux[core]["idx"] = idx2
    return f_hosts, aux


# ---------------- v9: v7 retuned (no warmup, fewer chunks, tail options) ----------------


def build_program_v9(iters: int = 1, dt=F16, psbufs: int = 4, obufs: int = 3,
                     groups=(5, 5, 5, 5, 5), bsplit=(2, 3, 4, 8, 8),
                     fsplit=(9, 10, 10), warm: int = 0):
    """v7 with chunk counts tuned for the 11-semaphore DMA pool: 5 band
    chunks + 3 feature chunks issue up-front without semaphore recycling
    stalls; outputs FIFO behind the band on the sync queue."""
    nc = bacc.Bacc(None, target_bir_lowering=False, debug=False)
    f_in = nc.dram_tensor("f", [128, NR, C], dt, kind="ExternalInput")
    b_in = nc.dram_tensor("b", [128, HC, KS, NCOL], dt, kind="ExternalInput")
    out = nc.dram_tensor("out", [C, S * HC, S * W], F16, kind="ExternalOutput")

    with tile.TileContext(nc) as tc:
        with (
            tc.tile_pool(name="fsb", bufs=1) as fpool,
            tc.tile_pool(name="bsb", bufs=1) as bpool,
            tc.tile_pool(name="osb", bufs=obufs) as opool,
            tc.tile_pool(name="ps0", bufs=psbufs, space="PSUM") as pspool0,
            tc.tile_pool(name="ps1", bufs=psbufs, space="PSUM") as pspool1,
        ):
            pspools = [pspool0, pspool1]

            def body(_=None):
                F_sb = fpool.tile([128, NR, C], dt)
                B_sb = bpool.tile([128, HC, KS, NCOL], dt)
                if warm == -1:
                    # interleave: scalar carries F plus every other B chunk
                    r0 = fsplit[0]
                    nc.scalar.dma_start(F_sb[:, :r0], f_in[:, :r0])
                    h0 = 0
                    fi = 1
                    for bi, bs in enumerate(bsplit):
                        eng = nc.scalar if bi % 2 == 1 else nc.sync
                        eng.dma_start(B_sb[:, h0 : h0 + bs], b_in[:, h0 : h0 + bs])
                        h0 += bs
                        if bi % 2 == 1 and fi < len(fsplit):
                            fs = fsplit[fi]
                            nc.scalar.dma_start(
                                F_sb[:, r0 : r0 + fs], f_in[:, r0 : r0 + fs]
                            )
                            r0 += fs
                            fi += 1
                else:
                    h0 = 0
                    for bs in bsplit:
                        nc.sync.dma_start(
                            B_sb[:, h0 : h0 + bs], b_in[:, h0 : h0 + bs]
                        )
                        h0 += bs
                    r0 = 0
                    for fs in fsplit:
                        nc.scalar.dma_start(
                            F_sb[:, r0 : r0 + fs], f_in[:, r0 : r0 + fs]
                        )
                        r0 += fs
                if warm > 0:
                    # keep the PE busy through the DMA lead-in so the HAM
                    # clock gate is at 2.4 GHz when real matmuls start.
                    # Sources read UNINITIALIZED F_sb rows (no producer dep,
                    # so the warm MMs start right after the preamble barrier;
                    # garbage values land in a scratch PSUM bank that real
                    # tiles later overwrite with start=True). The only side
                    # effect is a WAR dep deferring the last F chunk's issue,
                    # which lands long before its rows are consumed.
                    wps = pspool1.tile([128, 512], F32, name="warmps", tag="psb1")
                    wrhs = F_sb[:, NR - 2 : NR, :].rearrange("p r c -> p (r c)")
                    for k in range(warm):
                        nc.tensor.matmul(
                            wps[:, :], F_sb[:, NR - 1, 0:128], wrhs,
                            start=(k == 0), stop=(k == warm - 1),
                        )
                g0 = 0
                maxg = max(groups)
                for G in groups:
                    g1 = g0 + G
                    osb = [
                        opool.tile([128, maxg, 2, NBLK * S * L], F16,
                                   name=f"o{ct}_{g0}")
                        for ct in range(2)
                    ]
                    for h in range(g0, g1):
                        for ct in range(2):
                            ps = [
                                pspools[blk].tile(
                                    [128, NCOL], F32, name=f"ps{blk}_{ct}_{h}",
                                    tag=f"psb{blk}",
                                )
                                for blk in range(NBLK)
                            ]
                            for dy in range(KS):
                                for blk in range(NBLK):
                                    lo = KB2 * blk
                                    nc.tensor.matmul(
                                        ps[blk][:, :],
                                        F_sb[lo : lo + KB2, h + dy,
                                             ct * 128 : (ct + 1) * 128],
                                        B_sb[lo : lo + KB2, h, dy, :],
                                        start=(dy == 0),
                                        stop=(dy == KS - 1),
                                    )
                            dstv = osb[ct][:, h - g0].rearrange(
                                "p a (k w) -> p a k w", k=NBLK
                            )
                            for blk in range(NBLK):
                                src = ps[blk][:].rearrange("p (a w) -> p a w", a=2)
                                dst = dstv[:, :, blk, :]
                                if blk == 0:
                                    nc.vector.tensor_copy(dst, src)
                                else:
                                    nc.scalar.copy(dst, src)
                    for ct in range(2):
                        nc.sync.dma_start(
                            out[ct * 128 : (ct + 1) * 128, S * g0 : S * g1, :],
                            osb[ct][:, : g1 - g0].rearrange("p g a c -> p (g a c)"),
                        )
                    g0 = g1

            if iters == 1:
                body()
            else:
                with tc.For_i(0, iters, 1) as _i:
                    body(_i)
    nc.finalize()
    return nc


host_prep_v9 = host_prep_v2


# ---------------- v8: v7 + tiny lead chunks + PE warmup + finer out tail ----------------


def build_program_v8(iters: int = 1, dt=F16, psbufs: int = 4, obufs: int = 3,
                     groups=(5, 5, 5, 4, 3, 3), bsplit=(1, 2, 3, 4, 5, 5, 5),
                     fsplit=(5, 8, 8, 8), warm: int = 16):
    """v7 + (a) 1-row first band chunk so the first matmul fires ASAP,
    (b) dummy matmuls on scratch during the DMA lead-in to lift the PE HAM
    clock gate to 2.4 GHz before real work, (c) smaller trailing output
    groups so the final out-DMA after the last matmul is short."""
    nc = bacc.Bacc(None, target_bir_lowering=False, debug=False)
    f_in = nc.dram_tensor("f", [128, NR, C], dt, kind="ExternalInput")
    b_in = nc.dram_tensor("b", [128, HC, KS, NCOL], dt, kind="ExternalInput")
    out = nc.dram_tensor("out", [C, S * HC, S * W], F16, kind="ExternalOutput")

    with tile.TileContext(nc) as tc:
        with (
            tc.tile_pool(name="fsb", bufs=1) as fpool,
            tc.tile_pool(name="bsb", bufs=1) as bpool,
            tc.tile_pool(name="osb", bufs=obufs) as opool,
            tc.tile_pool(name="ps0", bufs=psbufs, space="PSUM") as pspool0,
            tc.tile_pool(name="ps1", bufs=psbufs, space="PSUM") as pspool1,
        ):
            pspools = [pspool0, pspool1]

            def body(_=None):
                F_sb = fpool.tile([128, NR, C], dt)
                B_sb = bpool.tile([128, HC, KS, NCOL], dt)
                if warm:
                    wsb = fpool.tile([128, 512], dt, name="wsb")
                    nc.vector.memset(wsb[:], 0.0)
                    wps = pspool0.tile([128, 512], F32, name="warmps", tag="psb0")
                    for k in range(warm):
                        nc.tensor.matmul(
                            wps[:, :], wsb[:, 0:128], wsb[:, :],
                            start=(k == 0), stop=(k == warm - 1),
                        )
                h0 = 0
                for bs in bsplit:
                    nc.sync.dma_start(B_sb[:, h0 : h0 + bs], b_in[:, h0 : h0 + bs])
                    h0 += bs
                r0 = 0
                for fs in fsplit:
                    nc.scalar.dma_start(F_sb[:, r0 : r0 + fs], f_in[:, r0 : r0 + fs])
                    r0 += fs
                g0 = 0
                for G in groups:
                    g1 = g0 + G
                    osb = [
                        opool.tile([128, groups[0], 2, NBLK * S * L], F16,
                                   name=f"o{ct}_{g0}")
                        for ct in range(2)
                    ]
                    for h in range(g0, g1):
                        for ct in range(2):
                            ps = [
                                pspools[blk].tile(
                                    [128, NCOL], F32, name=f"ps{blk}_{ct}_{h}",
                                    tag=f"psb{blk}",
                                )
                                for blk in range(NBLK)
                            ]
                            for dy in range(KS):
                                for blk in range(NBLK):
                                    lo = KB2 * blk
                                    nc.tensor.matmul(
                                        ps[blk][:, :],
                                        F_sb[lo : lo + KB2, h + dy,
                                             ct * 128 : (ct + 1) * 128],
                                        B_sb[lo : lo + KB2, h, dy, :],
                                        start=(dy == 0),
                                        stop=(dy == KS - 1),
                                    )
                            dstv = osb[ct][:, h - g0].rearrange(
                                "p a (k w) -> p a k w", k=NBLK
                            )
                            for blk in range(NBLK):
                                src = ps[blk][:].rearrange("p (a w) -> p a w", a=2)
                                dst = dstv[:, :, blk, :]
                                if blk == 0:
                                    nc.vector.tensor_copy(dst, src)
                                else:
                                    nc.scalar.copy(dst, src)
                    for ct in range(2):
                        nc.sync.dma_start(
                            out[ct * 128 : (ct + 1) * 128, S * g0 : S * g1, :],
                            osb[ct][:, : g1 - g0].rearrange("p g a c -> p (g a c)"),
                        )
                    g0 = g1

            if iters == 1:
                body()
            else:
                with tc.For_i(0, iters, 1) as _i:
                    body(_i)
    nc.finalize()
    return nc


host_prep_v8 = host_prep_v2


# ---------------- v6: on-device band build via GPSIMD local_scatter ----------------
NIDX = KS * KS * 2 * 2  # 100 scatter slots per partition: (dy, u=dx, a, b)


def build_program_v6(iters: int = 1, dt=F16, psbufs: int = 4, obufs: int = 3,
                     out_group: int = 5, dsplit=(5, 5, 5, 5, 5), fsplit=(9, 10, 10)):
    """v5 compute, but the banded mask matrices are built ON DEVICE:
    HBM carries only the compact masks pre-shifted per dx (d: 0.64 MB vs the
    6.4 MB host-built band) plus a 25.6 KB constant index table. One GPSIMD
    local_scatter per low-res row h places the 100 per-partition mask values
    onto the band diagonals (and zeroes the rest of that row's band slab)."""
    nc = bacc.Bacc(None, target_bir_lowering=False, debug=False)
    f_in = nc.dram_tensor("f", [128, NR, C], dt, kind="ExternalInput")
    d_in = nc.dram_tensor("d", [128, HC, NIDX], dt, kind="ExternalInput")
    ix_in = nc.dram_tensor("idx", [128, NIDX], mybir.dt.int16, kind="ExternalInput")
    out = nc.dram_tensor("out", [C, S * HC, S * W], F16, kind="ExternalOutput")

    with tile.TileContext(nc) as tc:
        with (
            tc.tile_pool(name="fsb", bufs=1) as fpool,
            tc.tile_pool(name="bsb", bufs=1) as bpool,
            tc.tile_pool(name="dsb", bufs=1) as dpool,
            tc.tile_pool(name="osb", bufs=obufs) as opool,
            tc.tile_pool(name="ps0", bufs=psbufs, space="PSUM") as pspool0,
            tc.tile_pool(name="ps1", bufs=psbufs, space="PSUM") as pspool1,
        ):
            pspools = [pspool0, pspool1]

            def body(_=None):
                F_sb = fpool.tile([128, NR, C], dt)
                B_sb = bpool.tile([128, HC, KS, NCOL], dt)
                D_sb = dpool.tile([128, HC, NIDX], dt)
                I_sb = dpool.tile([128, NIDX], mybir.dt.int16, name="isb")
                nc.sync.dma_start(I_sb[:], ix_in[:])
                h0 = 0
                for ds in dsplit:
                    nc.sync.dma_start(D_sb[:, h0 : h0 + ds], d_in[:, h0 : h0 + ds])
                    h0 += ds
                r0 = 0
                for fs in fsplit:
                    nc.scalar.dma_start(F_sb[:, r0 : r0 + fs], f_in[:, r0 : r0 + fs])
                    r0 += fs
                G = out_group
                for g0 in range(0, HC, G):
                    g1 = min(g0 + G, HC)
                    osb = [
                        opool.tile([128, G, 2, NBLK * S * L], F16, name=f"o{ct}_{g0}")
                        for ct in range(2)
                    ]
                    for h in range(g0, g1):
                        nc.gpsimd.local_scatter(
                            B_sb[:, h].rearrange("p d c -> p (d c)"),
                            D_sb[:, h],
                            I_sb[:],
                            channels=128,
                            num_elems=KS * NCOL,
                            num_idxs=NIDX,
                        )
                        for ct in range(2):
                            ps = [
                                pspools[blk].tile(
                                    [128, NCOL], F32, name=f"ps{blk}_{ct}_{h}",
                                    tag=f"psb{blk}",
                                )
                                for blk in range(NBLK)
                            ]
                            for dy in range(KS):
                                for blk in range(NBLK):
                                    lo = KB2 * blk
                                    nc.tensor.matmul(
                                        ps[blk][:, :],
                                        F_sb[lo : lo + KB2, h + dy,
                                             ct * 128 : (ct + 1) * 128],
                                        B_sb[lo : lo + KB2, h, dy, :],
                                        start=(dy == 0),
                                        stop=(dy == KS - 1),
                                    )
                            dstv = osb[ct][:, h - g0].rearrange(
                                "p a (k w) -> p a k w", k=NBLK
                            )
                            for blk in range(NBLK):
                                src = ps[blk][:].rearrange("p (a w) -> p a w", a=2)
                                dst = dstv[:, :, blk, :]
                                if blk == 0:
                                    nc.vector.tensor_copy(dst, src)
                                else:
                                    nc.scalar.copy(dst, src)
                    for ct in range(2):
                        nc.scalar.dma_start(
                            out[ct * 128 : (ct + 1) * 128, S * g0 : S * g1, :],
                            osb[ct][:, : g1 - g0].rearrange("p g a c -> p (g a c)"),
                        )

            if iters == 1:
                body()
            else:
                with tc.For_i(0, iters, 1) as _i:
                    body(_i)
    nc.finalize()
    return nc


def host_prep_v6(features: np.ndarray, masks: np.ndarray):
    """v6 inputs: F as in v2; d = per-dx partition-shifted compact masks
    [128, HC, (dy, u, a, b)]; idx = constant int16 scatter table [128, 100]."""
    f_hosts, _ = host_prep_v2(features, masks)
    d_hosts = []
    wl_idx = np.arange(L)
    for core in range(8):
        n, q = divmod(core, 4)
        h0 = HC * q
        m7 = masks[n].reshape(KS, KS, H, S, W, S)[:, :, h0 : h0 + HC]
        d_host = np.zeros((128, HC, KS, KS, 2, 2), np.float16)
        for blk in range(NBLK):
            for u in range(KS):
                src = m7[:, u, :, :, L * blk + wl_idx, :]  # [wl, dy, h, a, b]
                d_host[KB2 * blk + u + wl_idx, :, :, u, :, :] = (
                    src.transpose(0, 2, 1, 3, 4)
                )
        d_hosts.append(d_host.reshape(128, HC, NIDX))
    # scatter index table: for band partition q = 64*blk + p', slot (dy,u,a,b)
    # lands at free index ((dy*2 + a)*L + (p'-u))*2 + b of the (dy,a,wl,b) row,
    # or -1 (skipped) when wl = p'-u falls outside [0, L).
    idx = np.full((128, KS, KS, 2, 2), -1, np.int16)
    for blk in range(NBLK):
        for pp in range(KB):
            for u in range(KS):
                wl = pp - u
                if 0 <= wl < L:
                    for dy in range(KS):
                        for a in range(2):
                            for b in range(2):
                                idx[KB2 * blk + pp, dy, u, a, b] = (
                                    ((dy * 2 + a) * L + wl) * 2 + b
                                )
    idx_host = idx.reshape(128, NIDX)
    return f_hosts, [
        {"d": d_hosts[c], "idx": idx_host} for c in range(8)
    ]


# ---------------- v3: dy-pairs stacked in K (two taps per matmul) ----------------
NP3 = (KS + 1) // 2  # 3 matmuls per (h, blk): dy pairs (0,1), (2,3), (4,-)


def build_program_v3(iters: int = 1, dt=F16, copy_eng="both", parts="full",
                     psbufs: int = 3, obufs: int = 2, out_group: int = 5,
                     bchunks: int = 5, unroll: bool = False):
    """v3: K=128 = (dy-pair half j in {0,1}) x (w'' in [0,64)). The upper 64
    partitions hold a one-row-shifted copy of the features, so one matmul
    contracts two vertical taps. 300 matmuls of N=200, all tile_position
    (0,0), one PSUM bank per output row."""
    nc = bacc.Bacc(None, target_bir_lowering=False, debug=False)
    f_in = nc.dram_tensor("f", [128, NBLK, NR, C], dt, kind="ExternalInput")
    b_in = nc.dram_tensor("b", [128, NBLK, HC, NP3, NCOL], dt, kind="ExternalInput")
    out = nc.dram_tensor("out", [C, S * HC, S * W], F32, kind="ExternalOutput")

    with tile.TileContext(nc) as tc:
        with (
            tc.tile_pool(name="fsb", bufs=1) as fpool,
            tc.tile_pool(name="bsb", bufs=1) as bpool,
            tc.tile_pool(name="osb", bufs=obufs) as opool,
            tc.tile_pool(name="ps", bufs=psbufs, space="PSUM") as pspool,
        ):
            def body(_=None):
                F_sb = fpool.tile([128, NBLK, NR, C], dt)
                B_sb = bpool.tile([128, NBLK, HC, NP3, NCOL], dt)
                if parts != "nodmain":
                    nc.sync.dma_start(F_sb[:, :, : NR // 2], f_in[:, :, : NR // 2])
                    nc.sync.dma_start(F_sb[:, :, NR // 2 :], f_in[:, :, NR // 2 :])
                    bstep = (HC + bchunks - 1) // bchunks
                    for h0 in range(0, HC, bstep):
                        h1 = min(h0 + bstep, HC)
                        nc.sync.dma_start(B_sb[:, :, h0:h1], b_in[:, :, h0:h1])
                if parts == "dmain":
                    return
                G = out_group
                for ct in range(2):
                    for g0 in range(0, HC, G):
                        g1 = min(g0 + G, HC)
                        osb = opool.tile([128, G, 2, NBLK * S * L], F32)
                        for h in range(g0, g1):
                            ps = pspool.tile(
                                [128, NBLK * NCOL], F32, name=f"ps_{ct}_{h}",
                                tag="ps",
                            )
                            for blk in range(NBLK):
                                for p in range(NP3):
                                    nc.tensor.matmul(
                                        ps[:, blk * NCOL : (blk + 1) * NCOL],
                                        F_sb[:, blk, h + 2 * p,
                                             ct * 128 : (ct + 1) * 128],
                                        B_sb[:, blk, h, p, :],
                                        start=(blk == 0 and p == 0),
                                        stop=(blk == NBLK - 1 and p == NP3 - 1),
                                    )
                            if parts == "nocopy":
                                continue
                            # psum free layout (blk, a, w2l) -> dest (a, blk, w2l)
                            src = ps[:].rearrange("p (k a w) -> p a k w", k=NBLK, a=2)
                            dst = osb[:, h - g0].rearrange(
                                "p a (k w) -> p a k w", k=NBLK
                            )
                            if copy_eng == "vector" or (
                                copy_eng == "both" and h % 2 == 0
                            ):
                                nc.vector.tensor_copy(dst, src)
                            else:
                                nc.scalar.copy(dst, src)
                        if parts == "nocopy":
                            continue
                        nc.sync.dma_start(
                            out[ct * 128 : (ct + 1) * 128, S * g0 : S * g1, :],
                            osb[:, : g1 - g0].rearrange("p g a c -> p (g a c)"),
                        )

            if iters == 1:
                body()
            elif unroll:
                for _k in range(iters):
                    body(_k)
            else:
                with tc.For_i(0, iters, 1) as _i:
                    body(_i)
    nc.finalize()
    return nc


def host_prep_v3(features: np.ndarray, masks: np.ndarray):
    """v3 layouts: partition = 64*j + w''; j=1 half holds features shifted one
    row down (dy-pair trick). Separate windows per width block."""
    f_hosts, b_hosts = [], []
    padded = np.pad(features, ((0, 0), (0, 0), (R, R), (R, R)))
    wl_idx = np.arange(L)
    for core in range(8):
        n, q = divmod(core, 4)
        h0 = HC * q
        F_core = padded[n, :, h0 : h0 + NR, :]  # [C, 29, 104]
        F_host = np.zeros((128, NBLK, NR, C), np.float16)
        for blk in range(NBLK):
            win = F_core[:, :, L * blk : L * blk + KB].transpose(2, 1, 0)  # [54,29,C]
            F_host[:KB, blk] = win                      # j=0: rows r
            F_host[64 : 64 + KB, blk, : NR - 1] = win[:, 1:]  # j=1: rows r+1
        m7 = masks[n].reshape(KS, KS, H, S, W, S)[:, :, h0 : h0 + HC]
        B_host = np.zeros((128, NBLK, HC, NP3, 2, L, 2), np.float16)
        for blk in range(NBLK):
            for dx in range(KS):
                for dy in range(KS):
                    p, j = divmod(dy, 2)
                    src = m7[dy, dx, :, :, L * blk : L * blk + L, :]  # [h,a,wl,b]
                    B_host[64 * j + dx + wl_idx, blk, :, p, :, wl_idx, :] = (
                        src.transpose(2, 0, 1, 3)
                    )
        f_hosts.append(F_host)
        b_hosts.append(B_host.reshape(128, NBLK, HC, NP3, NCOL))
    return f_hosts, b_hosts


_NC_CACHE = {}

# Best HW-verified configuration (single-shot device exec ~51 us/core mean,
# ~56 us max across the 8 cores, vs ~76/82 us for the v2 baseline measured
# the same way):
#   - v2's banded-matmul compute (K=64, two concurrent PE row-group chains,
#     N=200 per matmul, PSUM-accumulated over the 5 vertical taps),
#   - fp16 output staging (halves the out-DMA to 5.12 MB/core; host upcasts),
#   - h-outer/ct-inner loop so each band chunk feeds 2x the matmuls,
#   - band chunks consumption-ordered on the sync HWDGE queue with a tiny
#     first chunk (first matmul fires ~10.5 us in), features on the scalar
#     queue, and all output DMAs FIFO'd on the sync queue BEHIND the band so
#     they can never starve the PE of mask data,
#   - finer trailing output groups to shorten the post-compute DMA tail.
_BEST_KW = dict(groups=(5, 5, 5, 5, 3, 2), bsplit=(1, 2, 4, 4, 7, 7),
                fsplit=(5, 9, 15))


def _get_program(iters: int = 1):
    if iters not in _NC_CACHE:
        _NC_CACHE[iters] = build_program_v9(iters, **_BEST_KW)
    return _NC_CACHE[iters]


def kernel(features: np.ndarray, masks: np.ndarray) -> np.ndarray:
    features = np.ascontiguousarray(features, dtype=np.float32)
    masks = np.ascontiguousarray(masks, dtype=np.float32)
    f_hosts, b_hosts = host_prep_v2(features, masks)
    in_maps = [{"f": f_hosts[c], "b": b_hosts[c]} for c in range(8)]
    nc = _get_program(1)
    res = run_bass_kernel_spmd(nc, in_maps, list(range(8)))
    out = np.empty((N, C, S * H, S * W), np.float32)
    for core in range(8):
        n, q = divmod(core, 4)
        out[n, :, S * HC * q : S * HC * (q + 1), :] = (
            res.results[core]["out"].astype(np.float32)
        )
    return out

